# revision 9
# baseline (speedup 1.0000x reference)
"""Trainium2 Bass kernel for a dense transformer block (pre-LN, causal MHA + FFN).

Sharding (8 NeuronCores): core c = 2*b + g handles sequence b (of B=4) and
half g (of 2): tensor-parallel attention over 8 of 16 heads (partial proj,
pairwise ReduceScatter over {2b, 2b+1}), then token-parallel LN2+FFN over
its 1024 of 2048 tokens. Device kernel works in transposed [C, T] layout;
host transposes in/out.

LayerNorm is folded into the matmuls: for Q^T = Wq^T @ LN(x)^T we accumulate
M = W~^T x plus rank-1 corrections (colsum(W~) (x) -mu + (beta@W) (x) std)
in PSUM, then scale columns by rstd at eviction. Same trick for the FFN
(relu is positively homogeneous, so rstd2 commutes out to the ff2 evict).

Matmul dtypes: float32r (full-rate fp32 variant, fed via casting DMAs) for
QKV/S/FFN-1; bf16 for attention V*P, proj, and FFN-2.
"""
import numpy as np
import ml_dtypes
from contextlib import ExitStack

B, T, C = 4, 2048, 1024
H, HS = 16, 64
F = 4 * C
P = 128
EPS = 1e-5
NCT = C // P        # 8 c-tiles
NFT = F // P        # 32 f-tiles
TL = T // 2         # 1024 local tokens
NPAIR = 4           # head-pairs per core
GROUPS = [[0, 1], [2, 3], [4, 5], [6, 7]]

_CACHE = {}


def _build(with_collective=True):
    import concourse.bass as bass
    import concourse.tile as tile
    from concourse import bacc, mybir

    f32 = mybir.dt.float32
    f32r = mybir.dt.float32r
    bf16 = mybir.dt.bfloat16
    AF = mybir.ActivationFunctionType
    OP = mybir.AluOpType

    nc = bacc.Bacc("TRN2", target_bir_lowering=False, debug=False, num_devices=8)

    # ---- DRAM I/O ----
    d_xT = nc.dram_tensor("xT", [NCT, P, T], f32, kind="ExternalInput").ap()
    d_xres = nc.dram_tensor("xresT", [NCT, P, TL], f32, kind="ExternalInput").ap()
    d_wqq = nc.dram_tensor("wqq", [NPAIR, NCT, P, P], f32, kind="ExternalInput").ap()
    d_wkk = nc.dram_tensor("wkk", [NPAIR, NCT, P, P], f32, kind="ExternalInput").ap()
    d_wvv = nc.dram_tensor("wvv", [NPAIR, NCT, P, P], f32, kind="ExternalInput").ap()
    d_ccq = nc.dram_tensor("ccq", [NPAIR, 2, P], f32, kind="ExternalInput").ap()
    d_cck = nc.dram_tensor("cck", [NPAIR, 2, P], f32, kind="ExternalInput").ap()
    d_ccv = nc.dram_tensor("ccv", [NPAIR, 2, P], f32, kind="ExternalInput").ap()
    d_wproj = nc.dram_tensor("wproj", [NPAIR, P, C], bf16, kind="ExternalInput").ap()
    d_bproj = nc.dram_tensor("bproj", [NCT, P], f32, kind="ExternalInput").ap()
    d_w1 = nc.dram_tensor("w1", [NCT, P, F], f32, kind="ExternalInput").ap()
    d_ccf = nc.dram_tensor("ccf", [NFT, 2, P], f32, kind="ExternalInput").ap()
    d_w2 = nc.dram_tensor("w2", [NFT, P, C], bf16, kind="ExternalInput").ap()
    d_b2 = nc.dram_tensor("b2", [NCT, P], f32, kind="ExternalInput").ap()
    d_m01 = nc.dram_tensor("m01", [4, P, 512], bf16, kind="ExternalInput").ap()
    d_ident = nc.dram_tensor("ident", [P, P], bf16, kind="ExternalInput").ap()
    d_ones = nc.dram_tensor("onesf", [1, T], f32, kind="ExternalInput").ap()
    d_out = nc.dram_tensor("outT", [NCT, P, TL], f32, kind="ExternalOutput").ap()

    with tile.TileContext(nc) as tc, ExitStack() as ctx:
        dram = ctx.enter_context(tc.tile_pool(name="dram", bufs=1, space="DRAM"))
        sa_bounce = dram.tile([2, NCT, P, TL], f32)
        sa_local = dram.tile([NCT, P, TL], f32)

        const = ctx.enter_context(tc.tile_pool(name="const", bufs=1))
        ones_bf = const.tile([P, 1], bf16)
        nc.vector.memset(ones_bf[:], 1.0)
        ident_bf = const.tile([P, P], bf16)
        nc.sync.dma_start(ident_bf[:], d_ident[:])
        masks = [const.tile([P, 512], bf16, name=f"mask{i}", tag=f"mask{i}")
                 for i in range(4)]
        for i in range(4):
            nc.sync.dma_start(masks[i][:], d_m01[i])
        ones1x64_r = const.tile([1, 64], f32r)
        nc.gpsimd.dma_start(ones1x64_r[:], d_ones[:, 0:64])
        onescol_r = const.tile([1, P], f32r)
        nc.gpsimd.dma_start(onescol_r[:], d_ones[:, 0:P])

        x2_dram = dram.tile([NCT, P, TL], f32)

        # long-lived pools, first-use pinned bottom-up so frees are LIFO.
        abc_pool = ctx.enter_context(tc.tile_pool(name="abc", bufs=1))
        a1bc = [abc_pool.tile([P, 512], f32, name=f"a1bc{ch}", tag=f"a1bc{ch}")
                for ch in range(4)]
        a2bc = [abc_pool.tile([P, 512], f32, name=f"a2bc{ch}", tag=f"a2bc{ch}")
                for ch in range(2)]
        for t_ in a1bc + a2bc:
            nc.vector.memset(t_[:, 0:1], 0.0)  # pin allocation order
        rowr_pool = ctx.enter_context(tc.tile_pool(name="rowr", bufs=1))
        rowr_pin = rowr_pool.tile([1, 1], f32, tag="pin")
        nc.vector.memset(rowr_pin[:], 0.0)
        pattr = ExitStack()
        attT_pool = pattr.enter_context(tc.tile_pool(name="attT", bufs=1))
        attT = [attT_pool.tile([P, T], bf16, name=f"attT{p}", tag=f"attT{p}")
                for p in range(NPAIR)]
        for p in range(NPAIR):
            nc.vector.memset(attT[p][:, 0:1], 0.0)
        pqkv = ExitStack()
        vaug_pool = pqkv.enter_context(tc.tile_pool(name="vaug", bufs=1))
        qq_pool = pqkv.enter_context(tc.tile_pool(name="qq", bufs=1))
        kk_pool = pqkv.enter_context(tc.tile_pool(name="kk", bufs=1))
        v_aug = {}
        for p in range(NPAIR):
            for st in range(16):
                va = vaug_pool.tile([P, 130], bf16, name=f"va{p}_{st}",
                                    tag=f"va{p}_{st}")
                nc.vector.memset(va[:, 64:65], 1.0)
                nc.vector.memset(va[:, 129:130], 1.0)
                v_aug[(p, st)] = va
        qq_r = [qq_pool.tile([P, T], f32r, name=f"qq{p}", tag=f"qq{p}")
                for p in range(NPAIR)]
        kk_r = [kk_pool.tile([P, T], f32r, name=f"kk{p}", tag=f"kk{p}")
                for p in range(NPAIR)]

        # =========== Phase 1: LN1 stats ===========
        p1 = ExitStack()
        xc_pool = p1.enter_context(tc.tile_pool(name="xc", bufs=4))
        bfc_pool = p1.enter_context(tc.tile_pool(name="bfc", bufs=6))
        rows1_pool = p1.enter_context(tc.tile_pool(name="rows1", bufs=4))
        stat_ps = p1.enter_context(tc.tile_pool(name="statps", bufs=2, space="PSUM"))
        bcp_ps = p1.enter_context(tc.tile_pool(name="bcpps", bufs=2, space="PSUM"))

        mu_row = rows1_pool.tile([1, T], f32, tag="row")
        ex2_row = rows1_pool.tile([1, T], f32, tag="row")
        for ch in range(4):
            sl = slice(ch * 512, (ch + 1) * 512)
            sx_ps = stat_ps.tile([1, 512], f32, tag="sx")
            sq_ps = stat_ps.tile([1, 512], f32, tag="sq")
            for ci in range(NCT):
                xc = xc_pool.tile([P, 512], f32, tag="xc")
                nc.sync.dma_start(xc[:], d_xT[ci][:, sl])
                xbfc = bfc_pool.tile([P, 512], bf16, tag="xbfc")
                nc.vector.tensor_copy(xbfc[:], xc[:])
                sqc = bfc_pool.tile([P, 512], bf16, tag="sqc")
                nc.vector.tensor_mul(sqc[:], xc[:], xc[:])
                nc.tensor.matmul(sx_ps[:], ones_bf[:], xbfc[:],
                                 start=(ci == 0), stop=(ci == NCT - 1))
                nc.tensor.matmul(sq_ps[:], ones_bf[:], sqc[:],
                                 start=(ci == 0), stop=(ci == NCT - 1))
            nc.scalar.mul(mu_row[:, sl], sx_ps[:], 1.0 / C)
            nc.scalar.mul(ex2_row[:, sl], sq_ps[:], 1.0 / C)

        var_row = rows1_pool.tile([1, T], f32, tag="row")
        nc.vector.tensor_mul(var_row[:], mu_row[:], mu_row[:])
        nc.vector.scalar_tensor_tensor(var_row[:], ex2_row[:], EPS,
                                       var_row[:], OP.add, OP.subtract)
        std_row = rows1_pool.tile([1, T], f32, tag="row")
        nc.scalar.activation(std_row[:], var_row[:], AF.Sqrt)
        rstd_row = rows1_pool.tile([1, T], f32, tag="row")
        nc.vector.reciprocal(rstd_row[:], std_row[:])
        nm_row = rows1_pool.tile([1, T], f32, tag="row")
        nc.scalar.mul(nm_row[:], mu_row[:], -1.0)

        rstd1_r = rowr_pool.tile([1, T], f32r, tag="rstdr")
        nc.gpsimd.dma_start(rstd1_r[:], rstd_row[:])
        xrow1_r = rowr_pool.tile([2, T], f32r, tag="xrowr")
        nc.gpsimd.dma_start(xrow1_r[0:1, :], nm_row[:])
        nc.gpsimd.dma_start(xrow1_r[1:2, :], std_row[:])

        for ch in range(4):
            sl = slice(ch * 512, (ch + 1) * 512)
            bc_ps = bcp_ps.tile([P, 512], f32, tag="bc")
            nc.tensor.matmul(bc_ps[:], onescol_r[:], rstd1_r[:, sl],
                             start=True, stop=True)
            nc.scalar.copy(a1bc[ch][:], bc_ps[:])
        p1.close()

        # =========== Phase 2: QKV per head-pair (LN folded) ===========
        p2 = ExitStack()
        w_pool = p2.enter_context(tc.tile_pool(name="wqkv", bufs=26))
        cc_pool = p2.enter_context(tc.tile_pool(name="cc", bufs=6))
        xr_pool = p2.enter_context(tc.tile_pool(name="xr", bufs=9))
        qk_ps = p2.enter_context(tc.tile_pool(name="qkps", bufs=2, space="PSUM"))
        ev_pool = p2.enter_context(tc.tile_pool(name="ev", bufs=3))
        tr_ps_pool = p2.enter_context(tc.tile_pool(name="trps", bufs=2, space="PSUM"))

        for p in range(NPAIR):
            wq_t, wk_t, wv_t = [], [], []
            for ci in range(NCT):
                wq = w_pool.tile([P, P], f32r, tag="w")
                nc.gpsimd.dma_start(wq[:], d_wqq[p, ci])
                wq_t.append(wq)
                wk = w_pool.tile([P, P], f32r, tag="w")
                nc.gpsimd.dma_start(wk[:], d_wkk[p, ci])
                wk_t.append(wk)
                wv = w_pool.tile([P, P], f32r, tag="w")
                nc.gpsimd.dma_start(wv[:], d_wvv[p, ci])
                wv_t.append(wv)
            ccq = cc_pool.tile([2, P], f32r, tag="cc")
            nc.gpsimd.dma_start(ccq[:], d_ccq[p])
            cck = cc_pool.tile([2, P], f32r, tag="cc")
            nc.gpsimd.dma_start(cck[:], d_cck[p])
            ccv = cc_pool.tile([2, P], f32r, tag="cc")
            nc.gpsimd.dma_start(ccv[:], d_ccv[p])
            for ch in range(4):
                sl = slice(ch * 512, (ch + 1) * 512)
                xrc = []
                for ci in range(NCT):
                    xr = xr_pool.tile([P, 512], f32r, tag="xr")
                    nc.gpsimd.dma_start(xr[:], d_xT[ci][:, sl])
                    xrc.append(xr)
                q_ps = qk_ps.tile([P, 512], f32, tag="q_ps")
                k_ps = qk_ps.tile([P, 512], f32, tag="k_ps")
                v_ps = qk_ps.tile([P, 512], f32, tag="v_ps")
                for ci in range(NCT):
                    nc.tensor.matmul(q_ps[:], wq_t[ci][:], xrc[ci][:], start=(ci == 0), stop=False)
                    nc.tensor.matmul(k_ps[:], wk_t[ci][:], xrc[ci][:], start=(ci == 0), stop=False)
                    nc.tensor.matmul(v_ps[:], wv_t[ci][:], xrc[ci][:], start=(ci == 0), stop=False)
                nc.tensor.matmul(q_ps[:], ccq[:], xrow1_r[:, sl], start=False, stop=True)
                nc.tensor.matmul(k_ps[:], cck[:], xrow1_r[:, sl], start=False, stop=True)
                nc.tensor.matmul(v_ps[:], ccv[:], xrow1_r[:, sl], start=False, stop=True)
                qev = ev_pool.tile([P, 512], f32, tag="ev")
                nc.vector.tensor_mul(qev[:], q_ps[:], a1bc[ch][:])
                nc.gpsimd.dma_start(qq_r[p][:, sl], qev[:])
                kev = ev_pool.tile([P, 512], f32, tag="ev")
                nc.vector.tensor_mul(kev[:], k_ps[:], a1bc[ch][:])
                nc.gpsimd.dma_start(kk_r[p][:, sl], kev[:])
                vev = ev_pool.tile([P, 512], bf16, tag="vev")
                nc.vector.tensor_mul(vev[:], v_ps[:], a1bc[ch][:])
                for sti in range(4):
                    st = ch * 4 + sti
                    for hh in range(2):
                        tp = tr_ps_pool.tile([P, 64], bf16, tag="tp")
                        nc.tensor.transpose(
                            tp[:], vev[hh * 64:(hh + 1) * 64, sti * 128:(sti + 1) * 128],
                            ident_bf[hh * 64:(hh + 1) * 64, hh * 64:(hh + 1) * 64])
                        nc.scalar.copy(v_aug[(p, st)][:, hh * 65:hh * 65 + 64], tp[:])
        p2.close()

        # =========== Phase 3: attention per head-pair ===========
        p3 = ExitStack()
        st_ps_pool = p3.enter_context(tc.tile_pool(name="stps", bufs=2, space="PSUM"))
        att_ps_pool = p3.enter_context(tc.tile_pool(name="attps", bufs=1, space="PSUM"))
        bc_ps_pool = p3.enter_context(tc.tile_pool(name="bcps", bufs=1, space="PSUM"))
        e_pool = p3.enter_context(tc.tile_pool(name="epool", bufs=6))
        rec_pool = p3.enter_context(tc.tile_pool(name="rec", bufs=2))
        recr_pool = p3.enter_context(tc.tile_pool(name="recr", bufs=2))
        bcsb_pool = p3.enter_context(tc.tile_pool(name="bcsb", bufs=2))

        for p in range(NPAIR):
            for qc in range(4):
                qsl = slice(qc * 512, (qc + 1) * 512)
                n_st = 4 * (qc + 1)
                attA = att_ps_pool.tile([65, 512], f32, tag="attA")
                attB = att_ps_pool.tile([65, 512], f32, tag="attB")
                for si in range(n_st):
                    ssl = slice(si * 128, (si + 1) * 128)
                    stA = st_ps_pool.tile([P, 512], f32, tag="stA")
                    stB = st_ps_pool.tile([P, 512], f32, tag="stB")
                    nc.tensor.matmul(stA[:], kk_r[p][0:64, ssl], qq_r[p][0:64, qsl],
                                     start=True, stop=True)
                    nc.tensor.matmul(stB[:], kk_r[p][64:128, ssl], qq_r[p][64:128, qsl],
                                     start=True, stop=True)
                    eA = e_pool.tile([P, 512], bf16, tag="e")
                    eB = e_pool.tile([P, 512], bf16, tag="e")
                    nc.scalar.activation(eA[:], stA[:], AF.Exp)
                    nc.scalar.activation(eB[:], stB[:], AF.Exp)
                    if si >= 4 * qc:
                        off = si - 4 * qc
                        nc.vector.tensor_mul(eA[:], eA[:], masks[off][:])
                        nc.vector.tensor_mul(eB[:], eB[:], masks[off][:])
                    nc.tensor.matmul(attA[:], v_aug[(p, si)][:, 0:65], eA[:],
                                     start=(si == 0), stop=(si == n_st - 1))
                    nc.tensor.matmul(attB[:], v_aug[(p, si)][:, 65:130], eB[:],
                                     start=(si == 0), stop=(si == n_st - 1))
                for hh, att in ((0, attA), (1, attB)):
                    rec = rec_pool.tile([1, 512], f32, tag="rec")
                    nc.vector.reciprocal(rec[:], att[64:65, :])
                    recr = recr_pool.tile([1, 512], f32r, tag="recr")
                    nc.gpsimd.dma_start(recr[:], rec[:])
                    bc_ps = bc_ps_pool.tile([64, 512], f32, tag="bc_ps")
                    nc.tensor.matmul(bc_ps[:], ones1x64_r[:], recr[:],
                                     start=True, stop=True)
                    bc_sb = bcsb_pool.tile([64, 512], f32, tag="bc_sb")
                    nc.scalar.copy(bc_sb[:], bc_ps[:])
                    nc.vector.tensor_mul(attT[p][hh * 64:(hh + 1) * 64, qsl],
                                         att[0:64, :], bc_sb[:])
        p3.close()
        pqkv.close()

        # =========== Phase 4: partial proj -> ReduceScatter -> x2 ===========
        p4 = ExitStack()
        wp_pool = p4.enter_context(tc.tile_pool(name="wproj", bufs=1))
        proj_ps = p4.enter_context(tc.tile_pool(name="projps", bufs=3, space="PSUM"))
        sa_pool = p4.enter_context(tc.tile_pool(name="sasb", bufs=4))
        xres_pool = p4.enter_context(tc.tile_pool(name="xres", bufs=2))
        bpj_pool = p4.enter_context(tc.tile_pool(name="bpj", bufs=1))

        bprojcol = [bpj_pool.tile([P, 1], f32, name=f"bpj{ci}", tag=f"bpj{ci}")
                    for ci in range(NCT)]
        for ci in range(NCT):
            nc.sync.dma_start(bprojcol[ci][:], d_bproj[ci].unsqueeze(1))
        wp = []
        for ki in range(NPAIR):
            w = wp_pool.tile([P, C], bf16, name=f"wp{ki}", tag=f"wp{ki}")
            nc.sync.dma_start(w[:], d_wproj[ki])
            wp.append(w)
        for co in range(NCT):
            for tc4 in range(4):
                sl = slice(tc4 * 512, (tc4 + 1) * 512)
                pp = proj_ps.tile([P, 512], f32, tag="pp")
                for ki in range(NPAIR):
                    nc.tensor.matmul(pp[:], wp[ki][:, co * P:(co + 1) * P],
                                     attT[ki][:, sl],
                                     start=(ki == 0), stop=(ki == NPAIR - 1))
                sa_sb = sa_pool.tile([P, 512], f32, tag="sa_sb")
                nc.scalar.copy(sa_sb[:], pp[:])
                fold, off = tc4 // 2, (tc4 % 2) * 512
                nc.sync.dma_start(sa_bounce[fold, co, :, off:off + 512], sa_sb[:])

        if with_collective:
            nc.gpsimd.collective_compute(
                "ReduceScatter",
                OP.add,
                replica_groups=GROUPS,
                ins=[sa_bounce.opt()],
                outs=[sa_local.opt()],
            )
        else:
            nc.sync.dma_start(sa_local[:], sa_bounce[0])

        for co in range(NCT):
            sal = xres_pool.tile([P, TL], f32, tag="sal")
            nc.sync.dma_start(sal[:], sa_local[co])
            xres = xres_pool.tile([P, TL], f32, tag="xres")
            nc.sync.dma_start(xres[:], d_xres[co])
            x2sb = xres_pool.tile([P, TL], f32, tag="x2sb")
            nc.vector.scalar_tensor_tensor(x2sb[:], sal[:], bprojcol[co][:],
                                           xres[:], OP.add, OP.add)
            nc.sync.dma_start(x2_dram[co], x2sb[:])
        p4.close()
        pattr.close()

        # =========== Phase 5: LN2 stats + x2 cast ===========
        x2r_pool = ctx.enter_context(tc.tile_pool(name="x2r", bufs=1))
        x2r = [x2r_pool.tile([P, TL], f32r, name=f"x2r{ci}", tag=f"x2r{ci}")
               for ci in range(NCT)]

        p5 = ExitStack()
        bfc2_pool = p5.enter_context(tc.tile_pool(name="bfc2", bufs=6))
        rows2_pool = p5.enter_context(tc.tile_pool(name="rows2", bufs=4))
        stat_ps2 = p5.enter_context(tc.tile_pool(name="statps2", bufs=2, space="PSUM"))
        bcp_ps2 = p5.enter_context(tc.tile_pool(name="bcpps2", bufs=2, space="PSUM"))

        mu2 = rows2_pool.tile([1, TL], f32, tag="row")
        ex22 = rows2_pool.tile([1, TL], f32, tag="row")
        for ch in range(2):
            sl = slice(ch * 512, (ch + 1) * 512)
            sx_ps = stat_ps2.tile([1, 512], f32, tag="sx")
            sq_ps = stat_ps2.tile([1, 512], f32, tag="sq")
            for ci in range(NCT):
                x2c = bfc2_pool.tile([P, 512], f32, tag="x2c")
                nc.sync.dma_start(x2c[:], x2_dram[ci][:, sl])
                xbfc = bfc2_pool.tile([P, 512], bf16, tag="xbfc")
                nc.vector.tensor_copy(xbfc[:], x2c[:])
                sqc = bfc2_pool.tile([P, 512], bf16, tag="sqc")
                nc.vector.tensor_mul(sqc[:], x2c[:], x2c[:])
                nc.tensor.matmul(sx_ps[:], ones_bf[:], xbfc[:],
                                 start=(ci == 0), stop=(ci == NCT - 1))
                nc.tensor.matmul(sq_ps[:], ones_bf[:], sqc[:],
                                 start=(ci == 0), stop=(ci == NCT - 1))
            nc.scalar.mul(mu2[:, sl], sx_ps[:], 1.0 / C)
            nc.scalar.mul(ex22[:, sl], sq_ps[:], 1.0 / C)
        var2 = rows2_pool.tile([1, TL], f32, tag="row")
        nc.vector.tensor_mul(var2[:], mu2[:], mu2[:])
        nc.vector.scalar_tensor_tensor(var2[:], ex22[:], EPS,
                                       var2[:], OP.add, OP.subtract)
        std2 = rows2_pool.tile([1, TL], f32, tag="row")
        nc.scalar.activation(std2[:], var2[:], AF.Sqrt)
        rstd2 = rows2_pool.tile([1, TL], f32, tag="row")
        nc.vector.reciprocal(rstd2[:], std2[:])
        nm2 = rows2_pool.tile([1, TL], f32, tag="row")
        nc.scalar.mul(nm2[:], mu2[:], -1.0)

        rstd2_r = rowr_pool.tile([1, TL], f32r, tag="rstdr")
        nc.gpsimd.dma_start(rstd2_r[:], rstd2[:])
        xrow2_r = rowr_pool.tile([2, TL], f32r, tag="xrowr")
        nc.gpsimd.dma_start(xrow2_r[0:1, :], nm2[:])
        nc.gpsimd.dma_start(xrow2_r[1:2, :], std2[:])

        for ch in range(2):
            sl = slice(ch * 512, (ch + 1) * 512)
            bc_ps = bcp_ps2.tile([P, 512], f32, tag="bc")
            nc.tensor.matmul(bc_ps[:], onescol_r[:], rstd2_r[:, sl],
                             start=True, stop=True)
            nc.scalar.copy(a2bc[ch][:], bc_ps[:])
        for ci in range(NCT):
            nc.gpsimd.dma_start(x2r[ci][:], x2_dram[ci])
        p5.close()

        # =========== Phase 6: FFN (LN folded, rstd2 deferred to ff2 evict) ===========
        p6 = ExitStack()
        w1_pool = p6.enter_context(tc.tile_pool(name="w1", bufs=24))
        ccf_pool = p6.enter_context(tc.tile_pool(name="ccf", bufs=4))
        w2_pool = p6.enter_context(tc.tile_pool(name="w2", bufs=48))
        ffn_ps = p6.enter_context(tc.tile_pool(name="ffnps", bufs=3, space="PSUM"))
        relu_pool = p6.enter_context(tc.tile_pool(name="relu", bufs=1))
        out_pool = p6.enter_context(tc.tile_pool(name="outsb", bufs=4))
        b2_pool = p6.enter_context(tc.tile_pool(name="b2p", bufs=1))

        b2col = [b2_pool.tile([P, 1], f32, name=f"b2c{ci}", tag=f"b2c{ci}")
                 for ci in range(NCT)]
        for ci in range(NCT):
            nc.sync.dma_start(b2col[ci][:], d_b2[ci].unsqueeze(1))

        relu1 = []
        for fo in range(NFT):
            rt = relu_pool.tile([P, TL], bf16, name=f"rl{fo}", tag=f"rl{fo}")
            relu1.append(rt)
            w1t = []
            for ci in range(NCT):
                w = w1_pool.tile([P, P], f32r, tag="w1t")
                nc.gpsimd.dma_start(w[:], d_w1[ci][:, fo * P:(fo + 1) * P])
                w1t.append(w)
            ccf = ccf_pool.tile([2, P], f32r, tag="ccf")
            nc.gpsimd.dma_start(ccf[:], d_ccf[fo])
            for tc2 in range(2):
                sl = slice(tc2 * 512, (tc2 + 1) * 512)
                fp = ffn_ps.tile([P, 512], f32, tag="fp")
                for ci in range(NCT):
                    nc.tensor.matmul(fp[:], w1t[ci][:], x2r[ci][:, sl], start=(ci == 0), stop=False)
                nc.tensor.matmul(fp[:], ccf[:], xrow2_r[:, sl], start=False, stop=True)
                nc.scalar.activation(rt[:, sl], fp[:], AF.Relu)

        for co in range(NCT):
            w2t = []
            for fi in range(NFT):
                w = w2_pool.tile([P, P], bf16, tag="w2t")
                nc.sync.dma_start(w[:], d_w2[fi][:, co * P:(co + 1) * P])
                w2t.append(w)
            for tc2 in range(2):
                sl = slice(tc2 * 512, (tc2 + 1) * 512)
                fp = ffn_ps.tile([P, 512], f32, tag="fp")
                for fi in range(NFT):
                    nc.tensor.matmul(fp[:], w2t[fi][:], relu1[fi][:, sl],
                                     start=(fi == 0), stop=(fi == NFT - 1))
                x2c = out_pool.tile([P, 512], f32, tag="x2c")
                nc.sync.dma_start(x2c[:], x2_dram[co][:, sl])
                tmp = out_pool.tile([P, 512], f32, tag="tmp")
                nc.vector.tensor_mul(tmp[:], fp[:], a2bc[tc2][:])
                osb = out_pool.tile([P, 512], f32, tag="osb")
                nc.vector.scalar_tensor_tensor(osb[:], tmp[:], b2col[co][:],
                                               x2c[:], OP.add, OP.add)
                nc.sync.dma_start(d_out[co][:, sl], osb[:])
        p6.close()

    nc.compile()
    return nc


def _prep_inputs(x, Wq, Wk, Wv, Wproj, bproj, W1, b1, W2, b2, g1, beta1, g2, beta2):
    """Build the 8 per-core input maps (host-side sharding + layout prep)."""
    f32 = np.float32
    scale = HS ** -0.5
    x = np.asarray(x, f32)
    Wq = np.asarray(Wq, f32)
    Wk = np.asarray(Wk, f32)
    Wv = np.asarray(Wv, f32)
    Wproj = np.asarray(Wproj, f32)
    W1 = np.asarray(W1, f32)
    b1 = np.asarray(b1, f32)
    W2 = np.asarray(W2, f32)
    g1 = np.asarray(g1, f32)
    beta1 = np.asarray(beta1, f32)
    g2 = np.asarray(g2, f32)
    beta2 = np.asarray(beta2, f32)

    w1g = g2[:, None] * W1
    b1p = b1 + beta2 @ W1
    ccf = np.stack([w1g.sum(0).reshape(NFT, P),
                    b1p.reshape(NFT, P)], axis=1)  # [NFT, 2, P]
    w2bf = W2.reshape(NFT, P, C).astype(ml_dtypes.bfloat16)
    b2r = np.asarray(b2, f32).reshape(NCT, P)
    bprojr = np.asarray(bproj, f32).reshape(NCT, P)
    sp = np.arange(P)[:, None]
    qf = np.arange(512)[None, :]
    m01 = np.stack([(sp + 128 * off <= qf) for off in range(4)]).astype(
        ml_dtypes.bfloat16)
    ident = np.eye(P, dtype=f32).astype(ml_dtypes.bfloat16)
    onesf = np.ones((1, T), f32)

    xT = [np.ascontiguousarray(x[b].T).reshape(NCT, P, T) for b in range(B)]

    def pair_weights(Wfull, g, scl):
        # raw pair weights [NPAIR, C, P]: cols 0:64 head g*8+2p, 64:128 head +1
        out = np.empty((NPAIR, C, P), f32)
        for p in range(NPAIR):
            hA, hB = g * 8 + 2 * p, g * 8 + 2 * p + 1
            out[p, :, 0:64] = Wfull[hA] * scl
            out[p, :, 64:128] = Wfull[hB] * scl
        return out

    per_g = {}
    for g in range(2):
        d = {}
        for nm, Wfull, scl in (("wqq", Wq, scale), ("wkk", Wk, 1.0), ("wvv", Wv, 1.0)):
            raw = pair_weights(Wfull, g, scl)      # [NPAIR, C, P]
            wt = g1[None, :, None] * raw           # g1-folded
            d[nm] = np.ascontiguousarray(wt).reshape(NPAIR, NCT, P, P)
            d["cc" + nm[1]] = np.stack(
                [wt.sum(1), np.einsum("c,pcd->pd", beta1, raw)], axis=1)  # [NPAIR,2,P]
        d["wproj"] = np.ascontiguousarray(
            Wproj[g * 512:(g + 1) * 512]).reshape(NPAIR, P, C).astype(
                ml_dtypes.bfloat16)
        per_g[g] = d

    in_maps = []
    for c in range(8):
        b, g = c // 2, c % 2
        m = {
            "xT": xT[b],
            "xresT": np.ascontiguousarray(xT[b][:, :, g * TL:(g + 1) * TL]),
            "bproj": bprojr,
            "w1": np.ascontiguousarray(w1g.reshape(NCT, P, F)),
            "ccf": ccf,
            "w2": w2bf,
            "b2": b2r,
            "m01": m01,
            "ident": ident,
            "onesf": onesf,
        }
        m.update(per_g[g])
        in_maps.append(m)
    return in_maps


def kernel(**inputs):
    from concourse.bass_utils import run_bass_kernel_spmd

    if "nc" not in _CACHE:
        _CACHE["nc"] = _build(with_collective=True)
    nc = _CACHE["nc"]
    in_maps = _prep_inputs(**inputs)
    res = run_bass_kernel_spmd(nc, in_maps, list(range(8)))
    out = np.empty((B, T, C), np.float32)
    for c in range(8):
        b, g = c // 2, c % 2
        outT = res.results[c]["outT"].reshape(C, TL)
        out[b, g * TL:(g + 1) * TL, :] = outT.T
    return out


# revision 11
# speedup vs baseline: 1.2747x; 1.2747x over previous
"""Trainium2 Bass kernel for a dense transformer block (pre-LN, causal MHA + FFN).

Sharding (8 NeuronCores): core c = 2*b + g handles sequence b (of B=4) and
half g (of 2): tensor-parallel attention over 8 of 16 heads (partial proj,
pairwise ReduceScatter over {2b, 2b+1}), then token-parallel LN2+FFN over
its 1024 of 2048 tokens. Device kernel works in transposed [C, T] layout;
host transposes in/out.

LayerNorm is folded into the matmuls: for Q^T = Wq^T @ LN(x)^T we accumulate
M = W~^T x plus rank-1 corrections (colsum(W~) (x) -mu + (beta@W) (x) std)
in PSUM, then scale columns by rstd at eviction. Same trick for the FFN
(relu is positively homogeneous, so rstd2 commutes out to the ff2 evict).

Matmul dtypes: float32r (full-rate fp32 variant, fed via casting DMAs) for
QKV/S/FFN-1; bf16 for attention V*P, proj, and FFN-2.
"""
import numpy as np
import ml_dtypes
from contextlib import ExitStack

B, T, C = 4, 2048, 1024
H, HS = 16, 64
F = 4 * C
P = 128
EPS = 1e-5
NCT = C // P        # 8 c-tiles
NFT = F // P        # 32 f-tiles
TL = T // 2         # 1024 local tokens
NPAIR = 4           # head-pairs per core
GROUPS = [[0, 1], [2, 3], [4, 5], [6, 7]]

_CACHE = {}


def _build(with_collective=True):
    import concourse.bass as bass
    import concourse.tile as tile
    from concourse import bacc, mybir

    f32 = mybir.dt.float32
    f32r = mybir.dt.float32r
    bf16 = mybir.dt.bfloat16
    AF = mybir.ActivationFunctionType
    OP = mybir.AluOpType

    nc = bacc.Bacc("TRN2", target_bir_lowering=False, debug=False, num_devices=8)

    # ---- DRAM I/O ----
    d_xT = nc.dram_tensor("xT", [NCT, P, T], f32, kind="ExternalInput").ap()
    d_xres = nc.dram_tensor("xresT", [NCT, P, TL], f32, kind="ExternalInput").ap()
    d_wqkv = nc.dram_tensor("wqkv", [NPAIR, NCT, P, 3 * P], f32,
                            kind="ExternalInput").ap()
    d_ccqkv = nc.dram_tensor("ccqkv", [NPAIR, 2, 3 * P], f32,
                             kind="ExternalInput").ap()
    d_wproj = nc.dram_tensor("wproj", [NPAIR, P, C], bf16, kind="ExternalInput").ap()
    d_bproj = nc.dram_tensor("bproj", [NCT, P], f32, kind="ExternalInput").ap()
    d_w1 = nc.dram_tensor("w1", [NCT, P, F], bf16, kind="ExternalInput").ap()
    d_ccf = nc.dram_tensor("ccf", [NFT, 2, P], bf16, kind="ExternalInput").ap()
    d_w2 = nc.dram_tensor("w2", [NFT, P, C], bf16, kind="ExternalInput").ap()
    d_b2 = nc.dram_tensor("b2", [NCT, P], f32, kind="ExternalInput").ap()
    d_m01 = nc.dram_tensor("m01", [4, P, 512], bf16, kind="ExternalInput").ap()
    d_ident = nc.dram_tensor("ident", [P, P], bf16, kind="ExternalInput").ap()
    d_out = nc.dram_tensor("outT", [NCT, P, TL], f32, kind="ExternalOutput").ap()

    with tile.TileContext(nc) as tc, ExitStack() as ctx:
        dram = ctx.enter_context(tc.tile_pool(name="dram", bufs=1, space="DRAM"))
        sa_bounce = dram.tile([2, NCT, P, TL], f32)
        sa_local = dram.tile([NCT, P, TL], f32)

        const = ctx.enter_context(tc.tile_pool(name="const", bufs=1))
        ones_bf = const.tile([P, 1], bf16)
        nc.vector.memset(ones_bf[:], 1.0)
        ident_bf = const.tile([P, P], bf16)
        nc.sync.dma_start(ident_bf[:], d_ident[:])
        masks = [const.tile([P, 512], bf16, name=f"mask{i}", tag=f"mask{i}")
                 for i in range(4)]
        for i in range(4):
            nc.sync.dma_start(masks[i][:], d_m01[i])
        ones1x64 = const.tile([1, 64], f32)
        nc.vector.memset(ones1x64[:], 1.0)
        onescol = const.tile([1, P], f32)
        nc.vector.memset(onescol[:], 1.0)

        x2_dram = dram.tile([NCT, P, TL], f32)

        # long-lived pools, first-use pinned bottom-up so frees are LIFO.
        abc_pool = ctx.enter_context(tc.tile_pool(name="abc", bufs=1))
        a1bc = [abc_pool.tile([P, 512], f32, name=f"a1bc{ch}", tag=f"a1bc{ch}")
                for ch in range(4)]
        a2bc = [abc_pool.tile([P, 512], f32, name=f"a2bc{ch}", tag=f"a2bc{ch}")
                for ch in range(2)]
        for t_ in a1bc + a2bc:
            nc.vector.memset(t_[:, 0:1], 0.0)  # pin allocation order
        rowr_pool = ctx.enter_context(tc.tile_pool(name="rowr", bufs=1))
        rowr_pin = rowr_pool.tile([1, 1], f32, tag="pin")
        nc.vector.memset(rowr_pin[:], 0.0)
        pattr = ExitStack()
        attT_pool = pattr.enter_context(tc.tile_pool(name="attT", bufs=1))
        attT = [attT_pool.tile([P, T], bf16, name=f"attT{p}", tag=f"attT{p}")
                for p in range(NPAIR)]
        for p in range(NPAIR):
            nc.vector.memset(attT[p][:, 0:1], 0.0)
        pqkv = ExitStack()
        vaug_pool = pqkv.enter_context(tc.tile_pool(name="vaug", bufs=1))
        qq_pool = pqkv.enter_context(tc.tile_pool(name="qq", bufs=1))
        kk_pool = pqkv.enter_context(tc.tile_pool(name="kk", bufs=1))
        v_aug = {}
        for p in range(NPAIR):
            for st in range(16):
                va = vaug_pool.tile([P, 130], bf16, name=f"va{p}_{st}",
                                    tag=f"va{p}_{st}")
                nc.vector.memset(va[:, 64:65], 1.0)
                nc.vector.memset(va[:, 129:130], 1.0)
                v_aug[(p, st)] = va
        qq_r = [qq_pool.tile([P, T], bf16, name=f"qq{p}", tag=f"qq{p}")
                for p in range(NPAIR)]
        kk_r = [kk_pool.tile([P, T], bf16, name=f"kk{p}", tag=f"kk{p}")
                for p in range(NPAIR)]

        # =========== Phase 1: LN1 stats ===========
        p1 = ExitStack()
        xc_pool = p1.enter_context(tc.tile_pool(name="xc", bufs=4))
        bfc_pool = p1.enter_context(tc.tile_pool(name="bfc", bufs=6))
        rows1_pool = p1.enter_context(tc.tile_pool(name="rows1", bufs=4))
        stat_ps = p1.enter_context(tc.tile_pool(name="statps", bufs=2, space="PSUM"))
        bcp_ps = p1.enter_context(tc.tile_pool(name="bcpps", bufs=2, space="PSUM"))

        mu_row = rows1_pool.tile([1, T], f32, tag="row")
        ex2_row = rows1_pool.tile([1, T], f32, tag="row")
        for ch in range(4):
            sl = slice(ch * 512, (ch + 1) * 512)
            sx_ps = stat_ps.tile([1, 512], f32, tag="sx")
            sq_ps = stat_ps.tile([1, 512], f32, tag="sq")
            for ci in range(NCT):
                xc = xc_pool.tile([P, 512], f32, tag="xc")
                nc.sync.dma_start(xc[:], d_xT[ci][:, sl])
                xbfc = bfc_pool.tile([P, 512], bf16, tag="xbfc")
                nc.vector.tensor_copy(xbfc[:], xc[:])
                sqc = bfc_pool.tile([P, 512], bf16, tag="sqc")
                nc.vector.tensor_mul(sqc[:], xc[:], xc[:])
                nc.tensor.matmul(sx_ps[:], ones_bf[:], xbfc[:],
                                 start=(ci == 0), stop=(ci == NCT - 1))
                nc.tensor.matmul(sq_ps[:], ones_bf[:], sqc[:],
                                 start=(ci == 0), stop=(ci == NCT - 1))
            nc.scalar.mul(mu_row[:, sl], sx_ps[:], 1.0 / C)
            nc.scalar.mul(ex2_row[:, sl], sq_ps[:], 1.0 / C)

        var_row = rows1_pool.tile([1, T], f32, tag="row")
        nc.vector.tensor_mul(var_row[:], mu_row[:], mu_row[:])
        nc.vector.scalar_tensor_tensor(var_row[:], ex2_row[:], EPS,
                                       var_row[:], OP.add, OP.subtract)
        std_row = rows1_pool.tile([1, T], f32, tag="row")
        nc.scalar.activation(std_row[:], var_row[:], AF.Sqrt)
        rstd_row = rows1_pool.tile([1, T], f32, tag="row")
        nc.vector.reciprocal(rstd_row[:], std_row[:])
        nm_row = rows1_pool.tile([1, T], f32, tag="row")
        nc.scalar.mul(nm_row[:], mu_row[:], -1.0)

        xrow1_r = rowr_pool.tile([2, T], f32r, tag="xrowr")
        nc.gpsimd.dma_start(xrow1_r[0:1, :], nm_row[:])
        nc.gpsimd.dma_start(xrow1_r[1:2, :], std_row[:])

        for ch in range(4):
            sl = slice(ch * 512, (ch + 1) * 512)
            bc_ps = bcp_ps.tile([P, 512], f32, tag="bc")
            nc.tensor.matmul(bc_ps[:], onescol[:], rstd_row[:, sl],
                             start=True, stop=True)
            nc.scalar.copy(a1bc[ch][:], bc_ps[:])
        p1.close()

        # =========== Phase 2: QKV per head-pair (LN folded) ===========
        p2 = ExitStack()
        w_pool = p2.enter_context(tc.tile_pool(name="wqkv", bufs=10))
        cc_pool = p2.enter_context(tc.tile_pool(name="cc", bufs=5))
        xr_pool = p2.enter_context(tc.tile_pool(name="xr", bufs=1))
        qk_ps = p2.enter_context(tc.tile_pool(name="qkps", bufs=2, space="PSUM"))
        ev_pool = p2.enter_context(tc.tile_pool(name="ev", bufs=3))
        tr_ps_pool = p2.enter_context(tc.tile_pool(name="trps", bufs=2, space="PSUM"))

        xr = [xr_pool.tile([P, T], f32r, name=f"xr{ci}", tag=f"xr{ci}")
              for ci in range(NCT)]
        for ci in range(NCT):
            nc.gpsimd.dma_start(xr[ci][:], d_xT[ci])
        for p in range(NPAIR):
            w_t = []
            for ci in range(NCT):
                w = w_pool.tile([P, 3 * P], f32r, tag="w")
                nc.gpsimd.dma_start(w[:], d_wqkv[p, ci])
                w_t.append(w)
            cc = cc_pool.tile([2, 3 * P], f32r, tag="cc")
            nc.gpsimd.dma_start(cc[:], d_ccqkv[p])
            for ch in range(4):
                sl = slice(ch * 512, (ch + 1) * 512)
                q_ps = qk_ps.tile([P, 512], f32, tag="q_ps")
                k_ps = qk_ps.tile([P, 512], f32, tag="k_ps")
                v_ps = qk_ps.tile([P, 512], f32, tag="v_ps")
                for ci in range(NCT):
                    nc.tensor.matmul(q_ps[:], w_t[ci][:, 0:P], xr[ci][:, sl],
                                     start=(ci == 0), stop=False)
                    nc.tensor.matmul(k_ps[:], w_t[ci][:, P:2 * P], xr[ci][:, sl],
                                     start=(ci == 0), stop=False)
                    nc.tensor.matmul(v_ps[:], w_t[ci][:, 2 * P:3 * P], xr[ci][:, sl],
                                     start=(ci == 0), stop=False)
                nc.tensor.matmul(q_ps[:], cc[:, 0:P], xrow1_r[:, sl],
                                 start=False, stop=True)
                nc.tensor.matmul(k_ps[:], cc[:, P:2 * P], xrow1_r[:, sl],
                                 start=False, stop=True)
                nc.tensor.matmul(v_ps[:], cc[:, 2 * P:3 * P], xrow1_r[:, sl],
                                 start=False, stop=True)
                nc.vector.tensor_mul(qq_r[p][:, sl], q_ps[:], a1bc[ch][:])
                nc.vector.tensor_mul(kk_r[p][:, sl], k_ps[:], a1bc[ch][:])
                vev = ev_pool.tile([P, 512], bf16, tag="vev")
                nc.vector.tensor_mul(vev[:], v_ps[:], a1bc[ch][:])
                for sti in range(4):
                    st = ch * 4 + sti
                    for hh in range(2):
                        tp = tr_ps_pool.tile([P, 64], bf16, tag="tp")
                        nc.tensor.transpose(
                            tp[:], vev[hh * 64:(hh + 1) * 64, sti * 128:(sti + 1) * 128],
                            ident_bf[hh * 64:(hh + 1) * 64, hh * 64:(hh + 1) * 64])
                        nc.scalar.copy(v_aug[(p, st)][:, hh * 65:hh * 65 + 64], tp[:])
        p2.close()

        # =========== Phase 3: attention per head-pair ===========
        p3 = ExitStack()
        st_ps_pool = p3.enter_context(tc.tile_pool(name="stps", bufs=2, space="PSUM"))
        att_ps_pool = p3.enter_context(tc.tile_pool(name="attps", bufs=1, space="PSUM"))
        bc_ps_pool = p3.enter_context(tc.tile_pool(name="bcps", bufs=1, space="PSUM"))
        e_pool = p3.enter_context(tc.tile_pool(name="epool", bufs=6))
        rec_pool = p3.enter_context(tc.tile_pool(name="rec", bufs=2))
        bcsb_pool = p3.enter_context(tc.tile_pool(name="bcsb", bufs=2))

        for p in range(NPAIR):
            for qc in range(4):
                qsl = slice(qc * 512, (qc + 1) * 512)
                n_st = 4 * (qc + 1)
                attA = att_ps_pool.tile([65, 512], f32, tag="attA")
                attB = att_ps_pool.tile([65, 512], f32, tag="attB")
                for si in range(n_st):
                    ssl = slice(si * 128, (si + 1) * 128)
                    stA = st_ps_pool.tile([P, 512], f32, tag="stA")
                    stB = st_ps_pool.tile([P, 512], f32, tag="stB")
                    nc.tensor.matmul(stA[:], kk_r[p][0:64, ssl], qq_r[p][0:64, qsl],
                                     start=True, stop=True)
                    nc.tensor.matmul(stB[:], kk_r[p][64:128, ssl], qq_r[p][64:128, qsl],
                                     start=True, stop=True)
                    eA = e_pool.tile([P, 512], bf16, tag="e")
                    eB = e_pool.tile([P, 512], bf16, tag="e")
                    nc.scalar.activation(eA[:], stA[:], AF.Exp)
                    nc.scalar.activation(eB[:], stB[:], AF.Exp)
                    if si >= 4 * qc:
                        off = si - 4 * qc
                        nc.vector.tensor_mul(eA[:], eA[:], masks[off][:])
                        nc.vector.tensor_mul(eB[:], eB[:], masks[off][:])
                    nc.tensor.matmul(attA[:], v_aug[(p, si)][:, 0:65], eA[:],
                                     start=(si == 0), stop=(si == n_st - 1))
                    nc.tensor.matmul(attB[:], v_aug[(p, si)][:, 65:130], eB[:],
                                     start=(si == 0), stop=(si == n_st - 1))
                for hh, att in ((0, attA), (1, attB)):
                    rec = rec_pool.tile([1, 512], f32, tag="rec")
                    nc.vector.reciprocal(rec[:], att[64:65, :])
                    bc_ps = bc_ps_pool.tile([64, 512], f32, tag="bc_ps")
                    nc.tensor.matmul(bc_ps[:], ones1x64[:], rec[:],
                                     start=True, stop=True)
                    bc_sb = bcsb_pool.tile([64, 512], f32, tag="bc_sb")
                    nc.scalar.copy(bc_sb[:], bc_ps[:])
                    nc.vector.tensor_mul(attT[p][hh * 64:(hh + 1) * 64, qsl],
                                         att[0:64, :], bc_sb[:])
        p3.close()
        pqkv.close()

        # =========== Phase 4: partial proj -> ReduceScatter -> x2 ===========
        p4 = ExitStack()
        wp_pool = p4.enter_context(tc.tile_pool(name="wproj", bufs=1))
        proj_ps = p4.enter_context(tc.tile_pool(name="projps", bufs=3, space="PSUM"))
        sa_pool = p4.enter_context(tc.tile_pool(name="sasb", bufs=4))
        xres_pool = p4.enter_context(tc.tile_pool(name="xres", bufs=2))
        bpj_pool = p4.enter_context(tc.tile_pool(name="bpj", bufs=1))

        bprojcol = [bpj_pool.tile([P, 1], f32, name=f"bpj{ci}", tag=f"bpj{ci}")
                    for ci in range(NCT)]
        for ci in range(NCT):
            nc.sync.dma_start(bprojcol[ci][:], d_bproj[ci].unsqueeze(1))
        wp = []
        for ki in range(NPAIR):
            w = wp_pool.tile([P, C], bf16, name=f"wp{ki}", tag=f"wp{ki}")
            nc.sync.dma_start(w[:], d_wproj[ki])
            wp.append(w)
        for co in range(NCT):
            for tc4 in range(4):
                sl = slice(tc4 * 512, (tc4 + 1) * 512)
                pp = proj_ps.tile([P, 512], f32, tag="pp")
                for ki in range(NPAIR):
                    nc.tensor.matmul(pp[:], wp[ki][:, co * P:(co + 1) * P],
                                     attT[ki][:, sl],
                                     start=(ki == 0), stop=(ki == NPAIR - 1))
                sa_sb = sa_pool.tile([P, 512], f32, tag="sa_sb")
                nc.scalar.copy(sa_sb[:], pp[:])
                fold, off = tc4 // 2, (tc4 % 2) * 512
                nc.sync.dma_start(sa_bounce[fold, co, :, off:off + 512], sa_sb[:])

        if with_collective:
            nc.gpsimd.collective_compute(
                "ReduceScatter",
                OP.add,
                replica_groups=GROUPS,
                ins=[sa_bounce.opt()],
                outs=[sa_local.opt()],
            )
        else:
            nc.sync.dma_start(sa_local[:], sa_bounce[0])

        for co in range(NCT):
            sal = xres_pool.tile([P, TL], f32, tag="sal")
            nc.sync.dma_start(sal[:], sa_local[co])
            xres = xres_pool.tile([P, TL], f32, tag="xres")
            nc.sync.dma_start(xres[:], d_xres[co])
            x2sb = xres_pool.tile([P, TL], f32, tag="x2sb")
            nc.vector.scalar_tensor_tensor(x2sb[:], sal[:], bprojcol[co][:],
                                           xres[:], OP.add, OP.add)
            nc.sync.dma_start(x2_dram[co], x2sb[:])
        p4.close()
        pattr.close()

        # =========== Phase 5: LN2 stats + x2 cast ===========
        x2r_pool = ctx.enter_context(tc.tile_pool(name="x2r", bufs=1))
        x2r = [x2r_pool.tile([P, TL], bf16, name=f"x2r{ci}", tag=f"x2r{ci}")
               for ci in range(NCT)]

        p5 = ExitStack()
        bfc2_pool = p5.enter_context(tc.tile_pool(name="bfc2", bufs=6))
        rows2_pool = p5.enter_context(tc.tile_pool(name="rows2", bufs=4))
        stat_ps2 = p5.enter_context(tc.tile_pool(name="statps2", bufs=2, space="PSUM"))
        bcp_ps2 = p5.enter_context(tc.tile_pool(name="bcpps2", bufs=2, space="PSUM"))

        mu2 = rows2_pool.tile([1, TL], f32, tag="row")
        ex22 = rows2_pool.tile([1, TL], f32, tag="row")
        for ch in range(2):
            sl = slice(ch * 512, (ch + 1) * 512)
            sx_ps = stat_ps2.tile([1, 512], f32, tag="sx")
            sq_ps = stat_ps2.tile([1, 512], f32, tag="sq")
            for ci in range(NCT):
                x2c = bfc2_pool.tile([P, 512], f32, tag="x2c")
                nc.sync.dma_start(x2c[:], x2_dram[ci][:, sl])
                xbfc = bfc2_pool.tile([P, 512], bf16, tag="xbfc")
                nc.vector.tensor_copy(xbfc[:], x2c[:])
                sqc = bfc2_pool.tile([P, 512], bf16, tag="sqc")
                nc.vector.tensor_mul(sqc[:], x2c[:], x2c[:])
                nc.tensor.matmul(sx_ps[:], ones_bf[:], xbfc[:],
                                 start=(ci == 0), stop=(ci == NCT - 1))
                nc.tensor.matmul(sq_ps[:], ones_bf[:], sqc[:],
                                 start=(ci == 0), stop=(ci == NCT - 1))
            nc.scalar.mul(mu2[:, sl], sx_ps[:], 1.0 / C)
            nc.scalar.mul(ex22[:, sl], sq_ps[:], 1.0 / C)
        var2 = rows2_pool.tile([1, TL], f32, tag="row")
        nc.vector.tensor_mul(var2[:], mu2[:], mu2[:])
        nc.vector.scalar_tensor_tensor(var2[:], ex22[:], EPS,
                                       var2[:], OP.add, OP.subtract)
        std2 = rows2_pool.tile([1, TL], f32, tag="row")
        nc.scalar.activation(std2[:], var2[:], AF.Sqrt)
        rstd2 = rows2_pool.tile([1, TL], f32, tag="row")
        nc.vector.reciprocal(rstd2[:], std2[:])
        nm2 = rows2_pool.tile([1, TL], f32, tag="row")
        nc.scalar.mul(nm2[:], mu2[:], -1.0)

        xrow2_r = rowr_pool.tile([2, TL], bf16, tag="xrow2b")
        nc.gpsimd.dma_start(xrow2_r[0:1, :], nm2[:])
        nc.gpsimd.dma_start(xrow2_r[1:2, :], std2[:])

        for ch in range(2):
            sl = slice(ch * 512, (ch + 1) * 512)
            bc_ps = bcp_ps2.tile([P, 512], f32, tag="bc")
            nc.tensor.matmul(bc_ps[:], onescol[:], rstd2[:, sl],
                             start=True, stop=True)
            nc.scalar.copy(a2bc[ch][:], bc_ps[:])
        for ci in range(NCT):
            nc.gpsimd.dma_start(x2r[ci][:], x2_dram[ci])
        p5.close()

        # =========== Phase 6: FFN (LN folded, rstd2 deferred to ff2 evict) ===========
        p6 = ExitStack()
        w1_pool = p6.enter_context(tc.tile_pool(name="w1", bufs=24))
        ccf_pool = p6.enter_context(tc.tile_pool(name="ccf", bufs=4))
        w2_pool = p6.enter_context(tc.tile_pool(name="w2", bufs=48))
        ffn_ps = p6.enter_context(tc.tile_pool(name="ffnps", bufs=3, space="PSUM"))
        relu_pool = p6.enter_context(tc.tile_pool(name="relu", bufs=1))
        out_pool = p6.enter_context(tc.tile_pool(name="outsb", bufs=4))
        b2_pool = p6.enter_context(tc.tile_pool(name="b2p", bufs=1))

        b2col = [b2_pool.tile([P, 1], f32, name=f"b2c{ci}", tag=f"b2c{ci}")
                 for ci in range(NCT)]
        for ci in range(NCT):
            nc.sync.dma_start(b2col[ci][:], d_b2[ci].unsqueeze(1))

        relu1 = []
        for fo in range(NFT):
            rt = relu_pool.tile([P, TL], bf16, name=f"rl{fo}", tag=f"rl{fo}")
            relu1.append(rt)
            w1t = []
            for ci in range(NCT):
                w = w1_pool.tile([P, P], bf16, tag="w1t")
                nc.sync.dma_start(w[:], d_w1[ci][:, fo * P:(fo + 1) * P])
                w1t.append(w)
            ccf = ccf_pool.tile([2, P], bf16, tag="ccf")
            nc.sync.dma_start(ccf[:], d_ccf[fo])
            for tc2 in range(2):
                sl = slice(tc2 * 512, (tc2 + 1) * 512)
                fp = ffn_ps.tile([P, 512], f32, tag="fp")
                for ci in range(NCT):
                    nc.tensor.matmul(fp[:], w1t[ci][:], x2r[ci][:, sl], start=(ci == 0), stop=False)
                nc.tensor.matmul(fp[:], ccf[:], xrow2_r[:, sl], start=False, stop=True)
                nc.scalar.activation(rt[:, sl], fp[:], AF.Relu)

        for co in range(NCT):
            w2t = []
            for fi in range(NFT):
                w = w2_pool.tile([P, P], bf16, tag="w2t")
                nc.sync.dma_start(w[:], d_w2[fi][:, co * P:(co + 1) * P])
                w2t.append(w)
            for tc2 in range(2):
                sl = slice(tc2 * 512, (tc2 + 1) * 512)
                fp = ffn_ps.tile([P, 512], f32, tag="fp")
                for fi in range(NFT):
                    nc.tensor.matmul(fp[:], w2t[fi][:], relu1[fi][:, sl],
                                     start=(fi == 0), stop=(fi == NFT - 1))
                x2c = out_pool.tile([P, 512], f32, tag="x2c")
                nc.sync.dma_start(x2c[:], x2_dram[co][:, sl])
                tmp = out_pool.tile([P, 512], f32, tag="tmp")
                nc.vector.tensor_mul(tmp[:], fp[:], a2bc[tc2][:])
                osb = out_pool.tile([P, 512], f32, tag="osb")
                nc.vector.scalar_tensor_tensor(osb[:], tmp[:], b2col[co][:],
                                               x2c[:], OP.add, OP.add)
                nc.sync.dma_start(d_out[co][:, sl], osb[:])
        p6.close()

    nc.compile()
    return nc


def _prep_inputs(x, Wq, Wk, Wv, Wproj, bproj, W1, b1, W2, b2, g1, beta1, g2, beta2):
    """Build the 8 per-core input maps (host-side sharding + layout prep)."""
    f32 = np.float32
    scale = HS ** -0.5
    x = np.asarray(x, f32)
    Wq = np.asarray(Wq, f32)
    Wk = np.asarray(Wk, f32)
    Wv = np.asarray(Wv, f32)
    Wproj = np.asarray(Wproj, f32)
    W1 = np.asarray(W1, f32)
    b1 = np.asarray(b1, f32)
    W2 = np.asarray(W2, f32)
    g1 = np.asarray(g1, f32)
    beta1 = np.asarray(beta1, f32)
    g2 = np.asarray(g2, f32)
    beta2 = np.asarray(beta2, f32)

    w1g = g2[:, None] * W1
    b1p = b1 + beta2 @ W1
    w1bf = w1g.reshape(NCT, P, F).astype(ml_dtypes.bfloat16)
    ccfbf = np.stack([w1g.sum(0).reshape(NFT, P),
                      b1p.reshape(NFT, P)], axis=1).astype(ml_dtypes.bfloat16)
    w2bf = W2.reshape(NFT, P, C).astype(ml_dtypes.bfloat16)
    b2r = np.asarray(b2, f32).reshape(NCT, P)
    bprojr = np.asarray(bproj, f32).reshape(NCT, P)
    sp = np.arange(P)[:, None]
    qf = np.arange(512)[None, :]
    m01 = np.stack([(sp + 128 * off <= qf) for off in range(4)]).astype(
        ml_dtypes.bfloat16)
    ident = np.eye(P, dtype=f32).astype(ml_dtypes.bfloat16)

    xT = [np.ascontiguousarray(x[b].T).reshape(NCT, P, T) for b in range(B)]

    def pair_weights(Wfull, g, scl):
        # raw pair weights [NPAIR, C, P]: cols 0:64 head g*8+2p, 64:128 head +1
        out = np.empty((NPAIR, C, P), f32)
        for p in range(NPAIR):
            hA, hB = g * 8 + 2 * p, g * 8 + 2 * p + 1
            out[p, :, 0:64] = Wfull[hA] * scl
            out[p, :, 64:128] = Wfull[hB] * scl
        return out

    per_g = {}
    for g in range(2):
        d = {}
        wqkv = np.empty((NPAIR, NCT, P, 3 * P), f32)
        ccqkv = np.empty((NPAIR, 2, 3 * P), f32)
        for j, (Wfull, scl) in enumerate(((Wq, scale), (Wk, 1.0), (Wv, 1.0))):
            raw = pair_weights(Wfull, g, scl)      # [NPAIR, C, P]
            wt = g1[None, :, None] * raw           # g1-folded
            wqkv[:, :, :, j * P:(j + 1) * P] = wt.reshape(NPAIR, NCT, P, P)
            ccqkv[:, 0, j * P:(j + 1) * P] = wt.sum(1)
            ccqkv[:, 1, j * P:(j + 1) * P] = np.einsum("c,pcd->pd", beta1, raw)
        d["wqkv"] = wqkv
        d["ccqkv"] = ccqkv
        d["wproj"] = np.ascontiguousarray(
            Wproj[g * 512:(g + 1) * 512]).reshape(NPAIR, P, C).astype(
                ml_dtypes.bfloat16)
        per_g[g] = d

    in_maps = []
    for c in range(8):
        b, g = c // 2, c % 2
        m = {
            "xT": xT[b],
            "xresT": np.ascontiguousarray(xT[b][:, :, g * TL:(g + 1) * TL]),
            "bproj": bprojr,
            "w1": w1bf,
            "ccf": ccfbf,
            "w2": w2bf,
            "b2": b2r,
            "m01": m01,
            "ident": ident,
        }
        m.update(per_g[g])
        in_maps.append(m)
    return in_maps


def kernel(**inputs):
    from concourse.bass_utils import run_bass_kernel_spmd

    if "nc" not in _CACHE:
        _CACHE["nc"] = _build(with_collective=True)
    nc = _CACHE["nc"]
    in_maps = _prep_inputs(**inputs)
    res = run_bass_kernel_spmd(nc, in_maps, list(range(8)))
    out = np.empty((B, T, C), np.float32)
    for c in range(8):
        b, g = c // 2, c % 2
        outT = res.results[c]["outT"].reshape(C, TL)
        out[b, g * TL:(g + 1) * TL, :] = outT.T
    return out


# revision 13
# speedup vs baseline: 1.5406x; 1.2086x over previous
"""Trainium2 Bass kernel for a dense transformer block (pre-LN, causal MHA + FFN).

Sharding (8 NeuronCores): core c = 2*b + g handles sequence b (of B=4) and
half g (of 2): tensor-parallel attention over 8 of 16 heads (partial proj,
pairwise ReduceScatter over {2b, 2b+1}), then token-parallel LN2+FFN over
its 1024 of 2048 tokens. Device kernel works in transposed [C, T] layout;
host transposes in/out.

LayerNorm is folded into the matmuls: for Q^T = Wq^T @ LN(x)^T we accumulate
M = W~^T x plus rank-1 corrections (colsum(W~) (x) -mu + (beta@W) (x) std)
in PSUM, then scale columns by rstd at eviction. Same trick for the FFN
(relu is positively homogeneous, so rstd2 commutes out to the ff2 evict).

Matmul dtypes: float32r (full-rate fp32 variant, fed via casting DMAs) for
QKV/S/FFN-1; bf16 for attention V*P, proj, and FFN-2.
"""
import numpy as np
import ml_dtypes
from contextlib import ExitStack

B, T, C = 4, 2048, 1024
H, HS = 16, 64
F = 4 * C
P = 128
EPS = 1e-5
NCT = C // P        # 8 c-tiles
NFT = F // P        # 32 f-tiles
TL = T // 2         # 1024 local tokens
NPAIR = 4           # head-pairs per core
GROUPS = [[0, 1], [2, 3], [4, 5], [6, 7]]

_CACHE = {}


def _build(with_collective=True):
    import concourse.bass as bass
    import concourse.tile as tile
    from concourse import bacc, mybir

    f32 = mybir.dt.float32
    f32r = mybir.dt.float32r
    bf16 = mybir.dt.bfloat16
    AF = mybir.ActivationFunctionType
    OP = mybir.AluOpType

    nc = bacc.Bacc("TRN2", target_bir_lowering=False, debug=False, num_devices=8)

    # ---- DRAM I/O ----
    d_xT = nc.dram_tensor("xT", [NCT, P, T], f32, kind="ExternalInput").ap()
    d_xres = nc.dram_tensor("xresT", [NCT, P, TL], f32, kind="ExternalInput").ap()
    d_wqkv = nc.dram_tensor("wqkv", [NPAIR, NCT, P, 3 * P], f32,
                            kind="ExternalInput").ap()
    d_ccqkv = nc.dram_tensor("ccqkv", [NPAIR, 2, 3 * P], f32,
                             kind="ExternalInput").ap()
    d_wproj = nc.dram_tensor("wproj", [NPAIR, P, C], bf16, kind="ExternalInput").ap()
    d_bproj = nc.dram_tensor("bproj", [NCT, P], f32, kind="ExternalInput").ap()
    d_w1 = nc.dram_tensor("w1", [NCT, P, F], bf16, kind="ExternalInput").ap()
    d_ccf = nc.dram_tensor("ccf", [NFT // 4, 2, 512], bf16, kind="ExternalInput").ap()
    d_w2 = nc.dram_tensor("w2", [NCT, NFT // 4, P, 512], bf16, kind="ExternalInput").ap()
    d_b2 = nc.dram_tensor("b2", [NCT, P], f32, kind="ExternalInput").ap()
    d_m01 = nc.dram_tensor("m01", [4, P, 512], bf16, kind="ExternalInput").ap()
    d_ident = nc.dram_tensor("ident", [P, P], bf16, kind="ExternalInput").ap()
    d_out = nc.dram_tensor("outT", [NCT, P, TL], f32, kind="ExternalOutput").ap()

    with tile.TileContext(nc) as tc, ExitStack() as ctx:
        dram = ctx.enter_context(tc.tile_pool(name="dram", bufs=1, space="DRAM"))
        sa_bounce = dram.tile([2, NCT, P, TL], f32)
        sa_local = dram.tile([NCT, P, TL], f32)

        const = ctx.enter_context(tc.tile_pool(name="const", bufs=1))
        ones_bf = const.tile([P, 1], bf16)
        nc.vector.memset(ones_bf[:], 1.0)
        ident_bf = const.tile([P, P], bf16)
        nc.sync.dma_start(ident_bf[:], d_ident[:])
        masks = [const.tile([P, 512], bf16, name=f"mask{i}", tag=f"mask{i}")
                 for i in range(4)]
        for i in range(4):
            nc.sync.dma_start(masks[i][:], d_m01[i])
        ones1x64 = const.tile([1, 64], f32)
        nc.vector.memset(ones1x64[:], 1.0)
        onescol = const.tile([1, P], f32)
        nc.vector.memset(onescol[:], 1.0)

        x2_dram = dram.tile([NCT, P, TL], f32)

        # long-lived pools, first-use pinned bottom-up so frees are LIFO.
        abc_pool = ctx.enter_context(tc.tile_pool(name="abc", bufs=1))
        a1bc = [abc_pool.tile([P, 512], f32, name=f"a1bc{ch}", tag=f"a1bc{ch}")
                for ch in range(4)]
        a2bc = [abc_pool.tile([P, 512], f32, name=f"a2bc{ch}", tag=f"a2bc{ch}")
                for ch in range(2)]
        for t_ in a1bc + a2bc:
            nc.vector.memset(t_[:, 0:1], 0.0)  # pin allocation order
        rowr_pool = ctx.enter_context(tc.tile_pool(name="rowr", bufs=1))
        rowr_pin = rowr_pool.tile([1, 1], f32, tag="pin")
        nc.vector.memset(rowr_pin[:], 0.0)
        pattr = ExitStack()
        attT_pool = pattr.enter_context(tc.tile_pool(name="attT", bufs=1))
        attT = [attT_pool.tile([P, T], bf16, name=f"attT{p}", tag=f"attT{p}")
                for p in range(NPAIR)]
        for p in range(NPAIR):
            nc.vector.memset(attT[p][:, 0:1], 0.0)
        pqkv = ExitStack()
        vaug_pool = pqkv.enter_context(tc.tile_pool(name="vaug", bufs=1))
        qq_pool = pqkv.enter_context(tc.tile_pool(name="qq", bufs=1))
        kk_pool = pqkv.enter_context(tc.tile_pool(name="kk", bufs=1))
        v_aug = {}
        for p in range(NPAIR):
            for st in range(16):
                va = vaug_pool.tile([P, 130], bf16, name=f"va{p}_{st}",
                                    tag=f"va{p}_{st}")
                nc.vector.memset(va[:, 64:65], 1.0)
                nc.vector.memset(va[:, 129:130], 1.0)
                v_aug[(p, st)] = va
        qq_r = [qq_pool.tile([P, T], bf16, name=f"qq{p}", tag=f"qq{p}")
                for p in range(NPAIR)]
        kk_r = [kk_pool.tile([P, T], bf16, name=f"kk{p}", tag=f"kk{p}")
                for p in range(NPAIR)]

        # =========== Phase 1: LN1 stats ===========
        p1 = ExitStack()
        xc_pool = p1.enter_context(tc.tile_pool(name="xc", bufs=4))
        bfc_pool = p1.enter_context(tc.tile_pool(name="bfc", bufs=6))
        rows1_pool = p1.enter_context(tc.tile_pool(name="rows1", bufs=4))
        stat_ps = p1.enter_context(tc.tile_pool(name="statps", bufs=2, space="PSUM"))
        bcp_ps = p1.enter_context(tc.tile_pool(name="bcpps", bufs=2, space="PSUM"))

        mu_row = rows1_pool.tile([1, T], f32, tag="row")
        ex2_row = rows1_pool.tile([1, T], f32, tag="row")
        for ch in range(4):
            sl = slice(ch * 512, (ch + 1) * 512)
            sx_ps = stat_ps.tile([1, 512], f32, tag="sx")
            sq_ps = stat_ps.tile([1, 512], f32, tag="sq")
            for ci in range(NCT):
                xc = xc_pool.tile([P, 512], f32, tag="xc")
                nc.sync.dma_start(xc[:], d_xT[ci][:, sl])
                xbfc = bfc_pool.tile([P, 512], bf16, tag="xbfc")
                nc.vector.tensor_copy(xbfc[:], xc[:])
                sqc = bfc_pool.tile([P, 512], bf16, tag="sqc")
                nc.vector.tensor_mul(sqc[:], xc[:], xc[:])
                nc.tensor.matmul(sx_ps[:], ones_bf[:], xbfc[:],
                                 start=(ci == 0), stop=(ci == NCT - 1))
                nc.tensor.matmul(sq_ps[:], ones_bf[:], sqc[:],
                                 start=(ci == 0), stop=(ci == NCT - 1))
            nc.scalar.mul(mu_row[:, sl], sx_ps[:], 1.0 / C)
            nc.scalar.mul(ex2_row[:, sl], sq_ps[:], 1.0 / C)

        var_row = rows1_pool.tile([1, T], f32, tag="row")
        nc.vector.tensor_mul(var_row[:], mu_row[:], mu_row[:])
        nc.vector.scalar_tensor_tensor(var_row[:], ex2_row[:], EPS,
                                       var_row[:], OP.add, OP.subtract)
        std_row = rows1_pool.tile([1, T], f32, tag="row")
        nc.scalar.activation(std_row[:], var_row[:], AF.Sqrt)
        rstd_row = rows1_pool.tile([1, T], f32, tag="row")
        nc.vector.reciprocal(rstd_row[:], std_row[:])
        nm_row = rows1_pool.tile([1, T], f32, tag="row")
        nc.scalar.mul(nm_row[:], mu_row[:], -1.0)

        xrow1_r = rowr_pool.tile([2, T], f32r, tag="xrowr")
        nc.gpsimd.dma_start(xrow1_r[0:1, :], nm_row[:])
        nc.gpsimd.dma_start(xrow1_r[1:2, :], std_row[:])

        for ch in range(4):
            sl = slice(ch * 512, (ch + 1) * 512)
            bc_ps = bcp_ps.tile([P, 512], f32, tag="bc")
            nc.tensor.matmul(bc_ps[:], onescol[:], rstd_row[:, sl],
                             start=True, stop=True)
            nc.scalar.copy(a1bc[ch][:], bc_ps[:])
        p1.close()

        # =========== Phase 2: QKV per head-pair (LN folded) ===========
        p2 = ExitStack()
        w_pool = p2.enter_context(tc.tile_pool(name="wqkv", bufs=10))
        cc_pool = p2.enter_context(tc.tile_pool(name="cc", bufs=5))
        xr_pool = p2.enter_context(tc.tile_pool(name="xr", bufs=1))
        qk_ps = p2.enter_context(tc.tile_pool(name="qkps", bufs=2, space="PSUM"))
        ev_pool = p2.enter_context(tc.tile_pool(name="ev", bufs=3))
        tr_ps_pool = p2.enter_context(tc.tile_pool(name="trps", bufs=2, space="PSUM"))

        xr = [xr_pool.tile([P, T], f32r, name=f"xr{ci}", tag=f"xr{ci}")
              for ci in range(NCT)]
        for ci in range(NCT):
            nc.gpsimd.dma_start(xr[ci][:], d_xT[ci])
        for p in range(NPAIR):
            w_t = []
            for ci in range(NCT):
                w = w_pool.tile([P, 3 * P], f32r, tag="w")
                nc.gpsimd.dma_start(w[:], d_wqkv[p, ci])
                w_t.append(w)
            cc = cc_pool.tile([2, 3 * P], f32r, tag="cc")
            nc.gpsimd.dma_start(cc[:], d_ccqkv[p])
            for ch in range(4):
                sl = slice(ch * 512, (ch + 1) * 512)
                q_ps = qk_ps.tile([P, 512], f32, tag="q_ps")
                k_ps = qk_ps.tile([P, 512], f32, tag="k_ps")
                v_ps = qk_ps.tile([P, 512], f32, tag="v_ps")
                for ci in range(NCT):
                    nc.tensor.matmul(q_ps[:], w_t[ci][:, 0:P], xr[ci][:, sl],
                                     start=(ci == 0), stop=False)
                    nc.tensor.matmul(k_ps[:], w_t[ci][:, P:2 * P], xr[ci][:, sl],
                                     start=(ci == 0), stop=False)
                    nc.tensor.matmul(v_ps[:], w_t[ci][:, 2 * P:3 * P], xr[ci][:, sl],
                                     start=(ci == 0), stop=False)
                nc.tensor.matmul(q_ps[:], cc[:, 0:P], xrow1_r[:, sl],
                                 start=False, stop=True)
                nc.tensor.matmul(k_ps[:], cc[:, P:2 * P], xrow1_r[:, sl],
                                 start=False, stop=True)
                nc.tensor.matmul(v_ps[:], cc[:, 2 * P:3 * P], xrow1_r[:, sl],
                                 start=False, stop=True)
                nc.vector.tensor_mul(qq_r[p][:, sl], q_ps[:], a1bc[ch][:])
                nc.vector.tensor_mul(kk_r[p][:, sl], k_ps[:], a1bc[ch][:])
                vev = ev_pool.tile([P, 512], bf16, tag="vev")
                nc.vector.tensor_mul(vev[:], v_ps[:], a1bc[ch][:])
                for sti in range(4):
                    st = ch * 4 + sti
                    for hh in range(2):
                        tp = tr_ps_pool.tile([P, 64], bf16, tag="tp")
                        nc.tensor.transpose(
                            tp[:], vev[hh * 64:(hh + 1) * 64, sti * 128:(sti + 1) * 128],
                            ident_bf[hh * 64:(hh + 1) * 64, hh * 64:(hh + 1) * 64])
                        nc.scalar.copy(v_aug[(p, st)][:, hh * 65:hh * 65 + 64], tp[:])
        p2.close()

        # =========== Phase 3: attention per head-pair ===========
        p3 = ExitStack()
        st_ps_pool = p3.enter_context(tc.tile_pool(name="stps", bufs=2, space="PSUM"))
        att_ps_pool = p3.enter_context(tc.tile_pool(name="attps", bufs=1, space="PSUM"))
        bc_ps_pool = p3.enter_context(tc.tile_pool(name="bcps", bufs=1, space="PSUM"))
        e_pool = p3.enter_context(tc.tile_pool(name="epool", bufs=6))
        rec_pool = p3.enter_context(tc.tile_pool(name="rec", bufs=2))
        bcsb_pool = p3.enter_context(tc.tile_pool(name="bcsb", bufs=2))

        for p in range(NPAIR):
            for qc in range(4):
                qsl = slice(qc * 512, (qc + 1) * 512)
                n_st = 4 * (qc + 1)
                attA = att_ps_pool.tile([65, 512], f32, tag="attA")
                attB = att_ps_pool.tile([65, 512], f32, tag="attB")
                for si in range(n_st):
                    ssl = slice(si * 128, (si + 1) * 128)
                    stA = st_ps_pool.tile([P, 512], f32, tag="stA")
                    stB = st_ps_pool.tile([P, 512], f32, tag="stB")
                    nc.tensor.matmul(stA[:], kk_r[p][0:64, ssl], qq_r[p][0:64, qsl],
                                     start=True, stop=True)
                    nc.tensor.matmul(stB[:], kk_r[p][64:128, ssl], qq_r[p][64:128, qsl],
                                     start=True, stop=True)
                    eA = e_pool.tile([P, 512], bf16, tag="e")
                    eB = e_pool.tile([P, 512], bf16, tag="e")
                    nc.scalar.activation(eA[:], stA[:], AF.Exp)
                    nc.scalar.activation(eB[:], stB[:], AF.Exp)
                    if si >= 4 * qc:
                        off = si - 4 * qc
                        nc.vector.tensor_mul(eA[:], eA[:], masks[off][:])
                        nc.vector.tensor_mul(eB[:], eB[:], masks[off][:])
                    nc.tensor.matmul(attA[:], v_aug[(p, si)][:, 0:65], eA[:],
                                     start=(si == 0), stop=(si == n_st - 1))
                    nc.tensor.matmul(attB[:], v_aug[(p, si)][:, 65:130], eB[:],
                                     start=(si == 0), stop=(si == n_st - 1))
                for hh, att in ((0, attA), (1, attB)):
                    rec = rec_pool.tile([1, 512], f32, tag="rec")
                    nc.vector.reciprocal(rec[:], att[64:65, :])
                    bc_ps = bc_ps_pool.tile([64, 512], f32, tag="bc_ps")
                    nc.tensor.matmul(bc_ps[:], ones1x64[:], rec[:],
                                     start=True, stop=True)
                    bc_sb = bcsb_pool.tile([64, 512], f32, tag="bc_sb")
                    nc.vector.tensor_copy(bc_sb[:], bc_ps[:])
                    nc.vector.tensor_mul(attT[p][hh * 64:(hh + 1) * 64, qsl],
                                         att[0:64, :], bc_sb[:])
        p3.close()
        pqkv.close()

        # =========== Phase 4: partial proj -> ReduceScatter -> x2 ===========
        p4 = ExitStack()
        wp_pool = p4.enter_context(tc.tile_pool(name="wproj", bufs=1))
        proj_ps = p4.enter_context(tc.tile_pool(name="projps", bufs=3, space="PSUM"))
        sa_pool = p4.enter_context(tc.tile_pool(name="sasb", bufs=4))
        xres_pool = p4.enter_context(tc.tile_pool(name="xres", bufs=2))
        bpj_pool = p4.enter_context(tc.tile_pool(name="bpj", bufs=1))

        bprojcol = [bpj_pool.tile([P, 1], f32, name=f"bpj{ci}", tag=f"bpj{ci}")
                    for ci in range(NCT)]
        for ci in range(NCT):
            nc.sync.dma_start(bprojcol[ci][:], d_bproj[ci].unsqueeze(1))
        wp = []
        for ki in range(NPAIR):
            w = wp_pool.tile([P, C], bf16, name=f"wp{ki}", tag=f"wp{ki}")
            nc.sync.dma_start(w[:], d_wproj[ki])
            wp.append(w)
        for co in range(NCT):
            for tc4 in range(4):
                sl = slice(tc4 * 512, (tc4 + 1) * 512)
                pp = proj_ps.tile([P, 512], f32, tag="pp")
                for ki in range(NPAIR):
                    nc.tensor.matmul(pp[:], wp[ki][:, co * P:(co + 1) * P],
                                     attT[ki][:, sl],
                                     start=(ki == 0), stop=(ki == NPAIR - 1))
                sa_sb = sa_pool.tile([P, 512], f32, tag="sa_sb")
                nc.vector.tensor_copy(sa_sb[:], pp[:])
                fold, off = tc4 // 2, (tc4 % 2) * 512
                nc.sync.dma_start(sa_bounce[fold, co, :, off:off + 512], sa_sb[:])

        if with_collective:
            nc.gpsimd.collective_compute(
                "ReduceScatter",
                OP.add,
                replica_groups=GROUPS,
                ins=[sa_bounce.opt()],
                outs=[sa_local.opt()],
            )
        else:
            nc.sync.dma_start(sa_local[:], sa_bounce[0])

        for co in range(NCT):
            sal = xres_pool.tile([P, TL], f32, tag="sal")
            nc.sync.dma_start(sal[:], sa_local[co])
            xres = xres_pool.tile([P, TL], f32, tag="xres")
            nc.sync.dma_start(xres[:], d_xres[co])
            x2sb = xres_pool.tile([P, TL], f32, tag="x2sb")
            nc.vector.scalar_tensor_tensor(x2sb[:], sal[:], bprojcol[co][:],
                                           xres[:], OP.add, OP.add)
            nc.sync.dma_start(x2_dram[co], x2sb[:])
        p4.close()
        pattr.close()

        # =========== Phase 5: LN2 stats + x2 cast ===========
        x2r_pool = ctx.enter_context(tc.tile_pool(name="x2r", bufs=1))
        x2r = [x2r_pool.tile([P, TL], bf16, name=f"x2r{ci}", tag=f"x2r{ci}")
               for ci in range(NCT)]

        p5 = ExitStack()
        bfc2_pool = p5.enter_context(tc.tile_pool(name="bfc2", bufs=6))
        rows2_pool = p5.enter_context(tc.tile_pool(name="rows2", bufs=4))
        stat_ps2 = p5.enter_context(tc.tile_pool(name="statps2", bufs=2, space="PSUM"))
        bcp_ps2 = p5.enter_context(tc.tile_pool(name="bcpps2", bufs=2, space="PSUM"))

        mu2 = rows2_pool.tile([1, TL], f32, tag="row")
        ex22 = rows2_pool.tile([1, TL], f32, tag="row")
        for ch in range(2):
            sl = slice(ch * 512, (ch + 1) * 512)
            sx_ps = stat_ps2.tile([1, 512], f32, tag="sx")
            sq_ps = stat_ps2.tile([1, 512], f32, tag="sq")
            for ci in range(NCT):
                x2c = bfc2_pool.tile([P, 512], f32, tag="x2c")
                nc.sync.dma_start(x2c[:], x2_dram[ci][:, sl])
                xbfc = bfc2_pool.tile([P, 512], bf16, tag="xbfc")
                nc.vector.tensor_copy(xbfc[:], x2c[:])
                sqc = bfc2_pool.tile([P, 512], bf16, tag="sqc")
                nc.vector.tensor_mul(sqc[:], x2c[:], x2c[:])
                nc.tensor.matmul(sx_ps[:], ones_bf[:], xbfc[:],
                                 start=(ci == 0), stop=(ci == NCT - 1))
                nc.tensor.matmul(sq_ps[:], ones_bf[:], sqc[:],
                                 start=(ci == 0), stop=(ci == NCT - 1))
            nc.scalar.mul(mu2[:, sl], sx_ps[:], 1.0 / C)
            nc.scalar.mul(ex22[:, sl], sq_ps[:], 1.0 / C)
        var2 = rows2_pool.tile([1, TL], f32, tag="row")
        nc.vector.tensor_mul(var2[:], mu2[:], mu2[:])
        nc.vector.scalar_tensor_tensor(var2[:], ex22[:], EPS,
                                       var2[:], OP.add, OP.subtract)
        std2 = rows2_pool.tile([1, TL], f32, tag="row")
        nc.scalar.activation(std2[:], var2[:], AF.Sqrt)
        rstd2 = rows2_pool.tile([1, TL], f32, tag="row")
        nc.vector.reciprocal(rstd2[:], std2[:])
        nm2 = rows2_pool.tile([1, TL], f32, tag="row")
        nc.scalar.mul(nm2[:], mu2[:], -1.0)

        xrow2_r = rowr_pool.tile([2, TL], bf16, tag="xrow2b")
        nc.gpsimd.dma_start(xrow2_r[0:1, :], nm2[:])
        nc.gpsimd.dma_start(xrow2_r[1:2, :], std2[:])

        for ch in range(2):
            sl = slice(ch * 512, (ch + 1) * 512)
            bc_ps = bcp_ps2.tile([P, 512], f32, tag="bc")
            nc.tensor.matmul(bc_ps[:], onescol[:], rstd2[:, sl],
                             start=True, stop=True)
            nc.scalar.copy(a2bc[ch][:], bc_ps[:])
        for ci in range(NCT):
            nc.gpsimd.dma_start(x2r[ci][:], x2_dram[ci])
        p5.close()

        # =========== Phase 6: FFN (LN folded, rstd2 deferred to ff2 evict) ===========
        p6 = ExitStack()
        w1_pool = p6.enter_context(tc.tile_pool(name="w1", bufs=12))
        ccf_pool = p6.enter_context(tc.tile_pool(name="ccf", bufs=4))
        w2_pool = p6.enter_context(tc.tile_pool(name="w2", bufs=12))
        ffn_ps = p6.enter_context(tc.tile_pool(name="ffnps", bufs=3, space="PSUM"))
        relu_pool = p6.enter_context(tc.tile_pool(name="relu", bufs=1))
        out_pool = p6.enter_context(tc.tile_pool(name="outsb", bufs=4))
        b2_pool = p6.enter_context(tc.tile_pool(name="b2p", bufs=1))

        b2col = [b2_pool.tile([P, 1], f32, name=f"b2c{ci}", tag=f"b2c{ci}")
                 for ci in range(NCT)]
        for ci in range(NCT):
            nc.sync.dma_start(b2col[ci][:], d_b2[ci].unsqueeze(1))

        relu1 = []
        for fog in range(NFT // 4):
            w1t = []
            for ci in range(NCT):
                w = w1_pool.tile([P, 512], bf16, tag="w1t")
                nc.sync.dma_start(w[:], d_w1[ci][:, fog * 512:(fog + 1) * 512])
                w1t.append(w)
            ccf = ccf_pool.tile([2, 512], bf16, tag="ccf")
            nc.sync.dma_start(ccf[:], d_ccf[fog])
            for fol in range(4):
                fo = fog * 4 + fol
                fsl = slice(fol * P, (fol + 1) * P)
                rt = relu_pool.tile([P, TL], bf16, name=f"rl{fo}", tag=f"rl{fo}")
                relu1.append(rt)
                for tc2 in range(2):
                    sl = slice(tc2 * 512, (tc2 + 1) * 512)
                    fp = ffn_ps.tile([P, 512], f32, tag="fp")
                    for ci in range(NCT):
                        nc.tensor.matmul(fp[:], w1t[ci][:, fsl], x2r[ci][:, sl],
                                         start=(ci == 0), stop=False)
                    nc.tensor.matmul(fp[:], ccf[:, fsl], xrow2_r[:, sl],
                                     start=False, stop=True)
                    nc.scalar.activation(rt[:, sl], fp[:], AF.Relu)

        for co in range(NCT):
            w2g = []
            for gq in range(NFT // 4):
                w = w2_pool.tile([P, 512], bf16, tag="w2t")
                nc.sync.dma_start(w[:], d_w2[co, gq])
                w2g.append(w)
            for tc2 in range(2):
                sl = slice(tc2 * 512, (tc2 + 1) * 512)
                fp = ffn_ps.tile([P, 512], f32, tag="fp")
                for fi in range(NFT):
                    nc.tensor.matmul(fp[:], w2g[fi // 4][:, (fi % 4) * P:(fi % 4 + 1) * P],
                                     relu1[fi][:, sl],
                                     start=(fi == 0), stop=(fi == NFT - 1))
                x2c = out_pool.tile([P, 512], f32, tag="x2c")
                nc.sync.dma_start(x2c[:], x2_dram[co][:, sl])
                tmp = out_pool.tile([P, 512], f32, tag="tmp")
                nc.vector.tensor_mul(tmp[:], fp[:], a2bc[tc2][:])
                osb = out_pool.tile([P, 512], f32, tag="osb")
                nc.vector.scalar_tensor_tensor(osb[:], tmp[:], b2col[co][:],
                                               x2c[:], OP.add, OP.add)
                nc.sync.dma_start(d_out[co][:, sl], osb[:])
        p6.close()

    nc.compile()
    return nc


def _prep_inputs(x, Wq, Wk, Wv, Wproj, bproj, W1, b1, W2, b2, g1, beta1, g2, beta2):
    """Build the 8 per-core input maps (host-side sharding + layout prep)."""
    f32 = np.float32
    scale = HS ** -0.5
    x = np.asarray(x, f32)
    Wq = np.asarray(Wq, f32)
    Wk = np.asarray(Wk, f32)
    Wv = np.asarray(Wv, f32)
    Wproj = np.asarray(Wproj, f32)
    W1 = np.asarray(W1, f32)
    b1 = np.asarray(b1, f32)
    W2 = np.asarray(W2, f32)
    g1 = np.asarray(g1, f32)
    beta1 = np.asarray(beta1, f32)
    g2 = np.asarray(g2, f32)
    beta2 = np.asarray(beta2, f32)

    w1g = g2[:, None] * W1
    b1p = b1 + beta2 @ W1
    w1bf = w1g.reshape(NCT, P, F).astype(ml_dtypes.bfloat16)
    ccfbf = np.stack([w1g.sum(0).reshape(NFT // 4, 512),
                      b1p.reshape(NFT // 4, 512)], axis=1).astype(ml_dtypes.bfloat16)
    # [NCT, NFT//4, P, 512]: w2h[co, gq, p, fi*128+cc] = W2[(gq*4+fi)*128+p, co*128+cc]
    w2bf = np.ascontiguousarray(
        W2.reshape(NFT // 4, 4, P, NCT, P).transpose(3, 0, 2, 1, 4).reshape(
            NCT, NFT // 4, P, 512)).astype(ml_dtypes.bfloat16)
    b2r = np.asarray(b2, f32).reshape(NCT, P)
    bprojr = np.asarray(bproj, f32).reshape(NCT, P)
    sp = np.arange(P)[:, None]
    qf = np.arange(512)[None, :]
    m01 = np.stack([(sp + 128 * off <= qf) for off in range(4)]).astype(
        ml_dtypes.bfloat16)
    ident = np.eye(P, dtype=f32).astype(ml_dtypes.bfloat16)

    xT = [np.ascontiguousarray(x[b].T).reshape(NCT, P, T) for b in range(B)]

    def pair_weights(Wfull, g, scl):
        # raw pair weights [NPAIR, C, P]: cols 0:64 head g*8+2p, 64:128 head +1
        out = np.empty((NPAIR, C, P), f32)
        for p in range(NPAIR):
            hA, hB = g * 8 + 2 * p, g * 8 + 2 * p + 1
            out[p, :, 0:64] = Wfull[hA] * scl
            out[p, :, 64:128] = Wfull[hB] * scl
        return out

    per_g = {}
    for g in range(2):
        d = {}
        wqkv = np.empty((NPAIR, NCT, P, 3 * P), f32)
        ccqkv = np.empty((NPAIR, 2, 3 * P), f32)
        for j, (Wfull, scl) in enumerate(((Wq, scale), (Wk, 1.0), (Wv, 1.0))):
            raw = pair_weights(Wfull, g, scl)      # [NPAIR, C, P]
            wt = g1[None, :, None] * raw           # g1-folded
            wqkv[:, :, :, j * P:(j + 1) * P] = wt.reshape(NPAIR, NCT, P, P)
            ccqkv[:, 0, j * P:(j + 1) * P] = wt.sum(1)
            ccqkv[:, 1, j * P:(j + 1) * P] = np.einsum("c,pcd->pd", beta1, raw)
        d["wqkv"] = wqkv
        d["ccqkv"] = ccqkv
        d["wproj"] = np.ascontiguousarray(
            Wproj[g * 512:(g + 1) * 512]).reshape(NPAIR, P, C).astype(
                ml_dtypes.bfloat16)
        per_g[g] = d

    in_maps = []
    for c in range(8):
        b, g = c // 2, c % 2
        m = {
            "xT": xT[b],
            "xresT": np.ascontiguousarray(xT[b][:, :, g * TL:(g + 1) * TL]),
            "bproj": bprojr,
            "w1": w1bf,
            "ccf": ccfbf,
            "w2": w2bf,
            "b2": b2r,
            "m01": m01,
            "ident": ident,
        }
        m.update(per_g[g])
        in_maps.append(m)
    return in_maps


def kernel(**inputs):
    from concourse.bass_utils import run_bass_kernel_spmd

    if "nc" not in _CACHE:
        _CACHE["nc"] = _build(with_collective=True)
    nc = _CACHE["nc"]
    in_maps = _prep_inputs(**inputs)
    res = run_bass_kernel_spmd(nc, in_maps, list(range(8)))
    out = np.empty((B, T, C), np.float32)
    for c in range(8):
        b, g = c // 2, c % 2
        outT = res.results[c]["outT"].reshape(C, TL)
        out[b, g * TL:(g + 1) * TL, :] = outT.T
    return out


# revision 14
# speedup vs baseline: 1.5818x; 1.0267x over previous
"""Trainium2 Bass kernel for a dense transformer block (pre-LN, causal MHA + FFN).

Sharding (8 NeuronCores): core c = 2*b + g handles sequence b (of B=4) and
half g (of 2): tensor-parallel attention over 8 of 16 heads (partial proj,
pairwise ReduceScatter over {2b, 2b+1}), then token-parallel LN2+FFN over
its 1024 of 2048 tokens. Device kernel works in transposed [C, T] layout;
host transposes in/out.

LayerNorm is folded into the matmuls: for Q^T = Wq^T @ LN(x)^T we accumulate
M = W~^T x plus rank-1 corrections (colsum(W~) (x) -mu + (beta@W) (x) std)
in PSUM, then scale columns by rstd at eviction. Same trick for the FFN
(relu is positively homogeneous, so rstd2 commutes out to the ff2 evict).

Matmul dtypes: float32r (full-rate fp32 variant, fed via casting DMAs) for
QKV/S/FFN-1; bf16 for attention V*P, proj, and FFN-2.
"""
import numpy as np
import ml_dtypes
from contextlib import ExitStack

B, T, C = 4, 2048, 1024
H, HS = 16, 64
F = 4 * C
P = 128
EPS = 1e-5
NCT = C // P        # 8 c-tiles
NFT = F // P        # 32 f-tiles
TL = T // 2         # 1024 local tokens
NPAIR = 4           # head-pairs per core
GROUPS = [[0, 1], [2, 3], [4, 5], [6, 7]]

_CACHE = {}


def _build(with_collective=True):
    import concourse.bass as bass
    import concourse.tile as tile
    from concourse import bacc, mybir

    f32 = mybir.dt.float32
    f32r = mybir.dt.float32r
    bf16 = mybir.dt.bfloat16
    AF = mybir.ActivationFunctionType
    OP = mybir.AluOpType

    nc = bacc.Bacc("TRN2", target_bir_lowering=False, debug=False, num_devices=8)

    # ---- DRAM I/O ----
    d_xT = nc.dram_tensor("xT", [NCT, P, T], f32, kind="ExternalInput").ap()
    d_xres = nc.dram_tensor("xresT", [NCT, P, TL], f32, kind="ExternalInput").ap()
    d_wqkv = nc.dram_tensor("wqkv", [NPAIR, NCT, P, 3 * P], f32,
                            kind="ExternalInput").ap()
    d_ccqkv = nc.dram_tensor("ccqkv", [NPAIR, 2, 3 * P], f32,
                             kind="ExternalInput").ap()
    d_wproj = nc.dram_tensor("wproj", [NPAIR, P, C], bf16, kind="ExternalInput").ap()
    d_bproj = nc.dram_tensor("bproj", [NCT, P], f32, kind="ExternalInput").ap()
    d_w1 = nc.dram_tensor("w1", [NCT, P, F], bf16, kind="ExternalInput").ap()
    d_ccf = nc.dram_tensor("ccf", [NFT // 4, 2, 512], bf16, kind="ExternalInput").ap()
    d_w2 = nc.dram_tensor("w2", [NCT, NFT // 4, P, 512], bf16, kind="ExternalInput").ap()
    d_b2 = nc.dram_tensor("b2", [NCT, P], f32, kind="ExternalInput").ap()
    d_m01 = nc.dram_tensor("m01", [4, P, 512], bf16, kind="ExternalInput").ap()
    d_ident = nc.dram_tensor("ident", [P, P], f32, kind="ExternalInput").ap()
    d_out = nc.dram_tensor("outT", [NCT, P, TL], f32, kind="ExternalOutput").ap()

    with tile.TileContext(nc) as tc, ExitStack() as ctx:
        dram = ctx.enter_context(tc.tile_pool(name="dram", bufs=1, space="DRAM"))
        sa_bounce = dram.tile([2, NCT, P, TL], f32)
        sa_local = dram.tile([NCT, P, TL], f32)

        const = ctx.enter_context(tc.tile_pool(name="const", bufs=1))
        ones_bf = const.tile([P, 1], bf16)
        nc.vector.memset(ones_bf[:], 1.0)
        ident_f32 = const.tile([P, P], f32)
        nc.sync.dma_start(ident_f32[:], d_ident[:])
        masks = [const.tile([P, 512], bf16, name=f"mask{i}", tag=f"mask{i}")
                 for i in range(4)]
        for i in range(4):
            nc.sync.dma_start(masks[i][:], d_m01[i])
        ones1x64 = const.tile([1, 64], f32)
        nc.vector.memset(ones1x64[:], 1.0)
        onescol = const.tile([1, P], f32)
        nc.vector.memset(onescol[:], 1.0)

        x2_dram = dram.tile([NCT, P, TL], f32)

        # long-lived pools, first-use pinned bottom-up so frees are LIFO.
        abc_pool = ctx.enter_context(tc.tile_pool(name="abc", bufs=1))
        a1bc = [abc_pool.tile([P, 512], f32, name=f"a1bc{ch}", tag=f"a1bc{ch}")
                for ch in range(4)]
        a2bc = [abc_pool.tile([P, 512], f32, name=f"a2bc{ch}", tag=f"a2bc{ch}")
                for ch in range(2)]
        for t_ in a1bc + a2bc:
            nc.vector.memset(t_[:, 0:1], 0.0)  # pin allocation order
        rowr_pool = ctx.enter_context(tc.tile_pool(name="rowr", bufs=1))
        rowr_pin = rowr_pool.tile([1, 1], f32, tag="pin")
        nc.vector.memset(rowr_pin[:], 0.0)
        pattr = ExitStack()
        attT_pool = pattr.enter_context(tc.tile_pool(name="attT", bufs=1))
        attT = [attT_pool.tile([P, T], bf16, name=f"attT{p}", tag=f"attT{p}")
                for p in range(NPAIR)]
        for p in range(NPAIR):
            nc.vector.memset(attT[p][:, 0:1], 0.0)
        pqkv = ExitStack()
        vaug_pool = pqkv.enter_context(tc.tile_pool(name="vaug", bufs=1))
        qq_pool = pqkv.enter_context(tc.tile_pool(name="qq", bufs=1))
        kk_pool = pqkv.enter_context(tc.tile_pool(name="kk", bufs=1))
        v_aug = {}
        for p in range(NPAIR):
            for st in range(16):
                va = vaug_pool.tile([P, 130], bf16, name=f"va{p}_{st}",
                                    tag=f"va{p}_{st}")
                nc.vector.memset(va[:, 64:65], 1.0)
                nc.vector.memset(va[:, 129:130], 1.0)
                v_aug[(p, st)] = va
        qq_r = [qq_pool.tile([P, T], bf16, name=f"qq{p}", tag=f"qq{p}")
                for p in range(NPAIR)]
        kk_r = [kk_pool.tile([P, T], bf16, name=f"kk{p}", tag=f"kk{p}")
                for p in range(NPAIR)]

        # =========== Phase 1: LN1 stats ===========
        p1 = ExitStack()
        xc_pool = p1.enter_context(tc.tile_pool(name="xc", bufs=4))
        bfc_pool = p1.enter_context(tc.tile_pool(name="bfc", bufs=6))
        rows1_pool = p1.enter_context(tc.tile_pool(name="rows1", bufs=4))
        stat_ps = p1.enter_context(tc.tile_pool(name="statps", bufs=2, space="PSUM"))
        bcp_ps = p1.enter_context(tc.tile_pool(name="bcpps", bufs=2, space="PSUM"))

        mu_row = rows1_pool.tile([1, T], f32, tag="row")
        ex2_row = rows1_pool.tile([1, T], f32, tag="row")
        for ch in range(4):
            sl = slice(ch * 512, (ch + 1) * 512)
            sx_ps = stat_ps.tile([1, 512], f32, tag="sx")
            sq_ps = stat_ps.tile([1, 512], f32, tag="sq")
            for ci in range(NCT):
                xc = xc_pool.tile([P, 512], f32, tag="xc")
                nc.sync.dma_start(xc[:], d_xT[ci][:, sl])
                xbfc = bfc_pool.tile([P, 512], bf16, tag="xbfc")
                nc.vector.tensor_copy(xbfc[:], xc[:])
                sqc = bfc_pool.tile([P, 512], bf16, tag="sqc")
                nc.vector.tensor_mul(sqc[:], xc[:], xc[:])
                nc.tensor.matmul(sx_ps[:], ones_bf[:], xbfc[:],
                                 start=(ci == 0), stop=(ci == NCT - 1))
                nc.tensor.matmul(sq_ps[:], ones_bf[:], sqc[:],
                                 start=(ci == 0), stop=(ci == NCT - 1))
            nc.scalar.mul(mu_row[:, sl], sx_ps[:], 1.0 / C)
            nc.scalar.mul(ex2_row[:, sl], sq_ps[:], 1.0 / C)

        var_row = rows1_pool.tile([1, T], f32, tag="row")
        nc.vector.tensor_mul(var_row[:], mu_row[:], mu_row[:])
        nc.vector.scalar_tensor_tensor(var_row[:], ex2_row[:], EPS,
                                       var_row[:], OP.add, OP.subtract)
        std_row = rows1_pool.tile([1, T], f32, tag="row")
        nc.scalar.activation(std_row[:], var_row[:], AF.Sqrt)
        rstd_row = rows1_pool.tile([1, T], f32, tag="row")
        nc.vector.reciprocal(rstd_row[:], std_row[:])
        nm_row = rows1_pool.tile([1, T], f32, tag="row")
        nc.scalar.mul(nm_row[:], mu_row[:], -1.0)

        xrow1_r = rowr_pool.tile([2, T], f32r, tag="xrowr")
        nc.gpsimd.dma_start(xrow1_r[0:1, :], nm_row[:])
        nc.gpsimd.dma_start(xrow1_r[1:2, :], std_row[:])

        for ch in range(4):
            sl = slice(ch * 512, (ch + 1) * 512)
            bc_ps = bcp_ps.tile([P, 512], f32, tag="bc")
            nc.tensor.matmul(bc_ps[:], onescol[:], rstd_row[:, sl],
                             start=True, stop=True)
            nc.scalar.copy(a1bc[ch][:], bc_ps[:])
        p1.close()

        # =========== Phase 2: QKV per head-pair (LN folded) ===========
        p2 = ExitStack()
        w_pool = p2.enter_context(tc.tile_pool(name="wqkv", bufs=10))
        cc_pool = p2.enter_context(tc.tile_pool(name="cc", bufs=5))
        xr_pool = p2.enter_context(tc.tile_pool(name="xr", bufs=1))
        qk_ps = p2.enter_context(tc.tile_pool(name="qkps", bufs=2, space="PSUM"))
        ev_pool = p2.enter_context(tc.tile_pool(name="ev", bufs=3))
        tr_ps_pool = p2.enter_context(tc.tile_pool(name="trps", bufs=2, space="PSUM"))

        xr = [xr_pool.tile([P, T], f32r, name=f"xr{ci}", tag=f"xr{ci}")
              for ci in range(NCT)]
        for ci in range(NCT):
            nc.gpsimd.dma_start(xr[ci][:], d_xT[ci])
        for p in range(NPAIR):
            w_t = []
            for ci in range(NCT):
                w = w_pool.tile([P, 3 * P], f32r, tag="w")
                nc.gpsimd.dma_start(w[:], d_wqkv[p, ci])
                w_t.append(w)
            cc = cc_pool.tile([2, 3 * P], f32r, tag="cc")
            nc.gpsimd.dma_start(cc[:], d_ccqkv[p])
            for ch in range(4):
                sl = slice(ch * 512, (ch + 1) * 512)
                q_ps = qk_ps.tile([P, 512], f32, tag="q_ps")
                k_ps = qk_ps.tile([P, 512], f32, tag="k_ps")
                v_ps = qk_ps.tile([P, 512], f32, tag="v_ps")
                for ci in range(NCT):
                    nc.tensor.matmul(q_ps[:], w_t[ci][:, 0:P], xr[ci][:, sl],
                                     start=(ci == 0), stop=False)
                    nc.tensor.matmul(k_ps[:], w_t[ci][:, P:2 * P], xr[ci][:, sl],
                                     start=(ci == 0), stop=False)
                    nc.tensor.matmul(v_ps[:], w_t[ci][:, 2 * P:3 * P], xr[ci][:, sl],
                                     start=(ci == 0), stop=False)
                nc.tensor.matmul(q_ps[:], cc[:, 0:P], xrow1_r[:, sl],
                                 start=False, stop=True)
                nc.tensor.matmul(k_ps[:], cc[:, P:2 * P], xrow1_r[:, sl],
                                 start=False, stop=True)
                nc.tensor.matmul(v_ps[:], cc[:, 2 * P:3 * P], xrow1_r[:, sl],
                                 start=False, stop=True)
                nc.vector.tensor_mul(qq_r[p][:, sl], q_ps[:], a1bc[ch][:])
                nc.vector.tensor_mul(kk_r[p][:, sl], k_ps[:], a1bc[ch][:])
                vev = ev_pool.tile([P, 512], f32, tag="vev")
                nc.vector.tensor_mul(vev[:], v_ps[:], a1bc[ch][:])
                for sti in range(4):
                    st = ch * 4 + sti
                    for hh in range(2):
                        tp = tr_ps_pool.tile([P, 64], f32, tag="tp")
                        nc.tensor.transpose(
                            tp[:], vev[hh * 64:(hh + 1) * 64, sti * 128:(sti + 1) * 128],
                            ident_f32[hh * 64:(hh + 1) * 64, hh * 64:(hh + 1) * 64])
                        nc.vector.tensor_copy(v_aug[(p, st)][:, hh * 65:hh * 65 + 64], tp[:])
        p2.close()

        # =========== Phase 3: attention per head-pair ===========
        p3 = ExitStack()
        st_ps_pool = p3.enter_context(tc.tile_pool(name="stps", bufs=2, space="PSUM"))
        att_ps_pool = p3.enter_context(tc.tile_pool(name="attps", bufs=1, space="PSUM"))
        e_pool = p3.enter_context(tc.tile_pool(name="epool", bufs=6))
        rec_pool = p3.enter_context(tc.tile_pool(name="rec", bufs=2))
        bcsb_pool = p3.enter_context(tc.tile_pool(name="bcsb", bufs=2))

        for p in range(NPAIR):
            for qc in range(4):
                qsl = slice(qc * 512, (qc + 1) * 512)
                n_st = 4 * (qc + 1)
                attA = att_ps_pool.tile([65, 512], f32, tag="attA")
                attB = att_ps_pool.tile([65, 512], f32, tag="attB")
                for si in range(n_st):
                    ssl = slice(si * 128, (si + 1) * 128)
                    stA = st_ps_pool.tile([P, 512], f32, tag="stA")
                    stB = st_ps_pool.tile([P, 512], f32, tag="stB")
                    nc.tensor.matmul(stA[:], kk_r[p][0:64, ssl], qq_r[p][0:64, qsl],
                                     start=True, stop=True)
                    nc.tensor.matmul(stB[:], kk_r[p][64:128, ssl], qq_r[p][64:128, qsl],
                                     start=True, stop=True)
                    eA = e_pool.tile([P, 512], bf16, tag="e")
                    eB = e_pool.tile([P, 512], bf16, tag="e")
                    nc.scalar.activation(eA[:], stA[:], AF.Exp)
                    nc.scalar.activation(eB[:], stB[:], AF.Exp)
                    if si >= 4 * qc:
                        off = si - 4 * qc
                        nc.vector.tensor_mul(eA[:], eA[:], masks[off][:])
                        nc.vector.tensor_mul(eB[:], eB[:], masks[off][:])
                    nc.tensor.matmul(attA[:], v_aug[(p, si)][:, 0:65], eA[:],
                                     start=(si == 0), stop=(si == n_st - 1))
                    nc.tensor.matmul(attB[:], v_aug[(p, si)][:, 65:130], eB[:],
                                     start=(si == 0), stop=(si == n_st - 1))
                for hh, att in ((0, attA), (1, attB)):
                    rec = rec_pool.tile([1, 512], f32, tag="rec")
                    nc.vector.reciprocal(rec[:], att[64:65, :])
                    bc_sb = bcsb_pool.tile([64, 512], f32, tag="bc_sb")
                    nc.gpsimd.partition_broadcast(bc_sb[:], rec[:])
                    nc.vector.tensor_mul(attT[p][hh * 64:(hh + 1) * 64, qsl],
                                         att[0:64, :], bc_sb[:])
        p3.close()
        pqkv.close()

        # =========== Phase 4: partial proj -> ReduceScatter -> x2 ===========
        p4 = ExitStack()
        wp_pool = p4.enter_context(tc.tile_pool(name="wproj", bufs=1))
        proj_ps = p4.enter_context(tc.tile_pool(name="projps", bufs=3, space="PSUM"))
        sa_pool = p4.enter_context(tc.tile_pool(name="sasb", bufs=4))
        xres_pool = p4.enter_context(tc.tile_pool(name="xres", bufs=2))
        bpj_pool = p4.enter_context(tc.tile_pool(name="bpj", bufs=1))

        bprojcol = [bpj_pool.tile([P, 1], f32, name=f"bpj{ci}", tag=f"bpj{ci}")
                    for ci in range(NCT)]
        for ci in range(NCT):
            nc.sync.dma_start(bprojcol[ci][:], d_bproj[ci].unsqueeze(1))
        wp = []
        for ki in range(NPAIR):
            w = wp_pool.tile([P, C], bf16, name=f"wp{ki}", tag=f"wp{ki}")
            nc.sync.dma_start(w[:], d_wproj[ki])
            wp.append(w)
        for co in range(NCT):
            for tc4 in range(4):
                sl = slice(tc4 * 512, (tc4 + 1) * 512)
                pp = proj_ps.tile([P, 512], f32, tag="pp")
                for ki in range(NPAIR):
                    nc.tensor.matmul(pp[:], wp[ki][:, co * P:(co + 1) * P],
                                     attT[ki][:, sl],
                                     start=(ki == 0), stop=(ki == NPAIR - 1))
                sa_sb = sa_pool.tile([P, 512], f32, tag="sa_sb")
                nc.vector.tensor_copy(sa_sb[:], pp[:])
                fold, off = tc4 // 2, (tc4 % 2) * 512
                nc.sync.dma_start(sa_bounce[fold, co, :, off:off + 512], sa_sb[:])

        if with_collective:
            nc.gpsimd.collective_compute(
                "ReduceScatter",
                OP.add,
                replica_groups=GROUPS,
                ins=[sa_bounce.opt()],
                outs=[sa_local.opt()],
            )
        else:
            nc.sync.dma_start(sa_local[:], sa_bounce[0])

        for co in range(NCT):
            sal = xres_pool.tile([P, TL], f32, tag="sal")
            nc.sync.dma_start(sal[:], sa_local[co])
            xres = xres_pool.tile([P, TL], f32, tag="xres")
            nc.sync.dma_start(xres[:], d_xres[co])
            x2sb = xres_pool.tile([P, TL], f32, tag="x2sb")
            nc.vector.scalar_tensor_tensor(x2sb[:], sal[:], bprojcol[co][:],
                                           xres[:], OP.add, OP.add)
            nc.sync.dma_start(x2_dram[co], x2sb[:])
        p4.close()
        pattr.close()

        # =========== Phase 5: LN2 stats + x2 cast ===========
        x2r_pool = ctx.enter_context(tc.tile_pool(name="x2r", bufs=1))
        x2r = [x2r_pool.tile([P, TL], bf16, name=f"x2r{ci}", tag=f"x2r{ci}")
               for ci in range(NCT)]

        p5 = ExitStack()
        bfc2_pool = p5.enter_context(tc.tile_pool(name="bfc2", bufs=6))
        rows2_pool = p5.enter_context(tc.tile_pool(name="rows2", bufs=4))
        stat_ps2 = p5.enter_context(tc.tile_pool(name="statps2", bufs=2, space="PSUM"))
        bcp_ps2 = p5.enter_context(tc.tile_pool(name="bcpps2", bufs=2, space="PSUM"))

        mu2 = rows2_pool.tile([1, TL], f32, tag="row")
        ex22 = rows2_pool.tile([1, TL], f32, tag="row")
        for ch in range(2):
            sl = slice(ch * 512, (ch + 1) * 512)
            sx_ps = stat_ps2.tile([1, 512], f32, tag="sx")
            sq_ps = stat_ps2.tile([1, 512], f32, tag="sq")
            for ci in range(NCT):
                x2c = bfc2_pool.tile([P, 512], f32, tag="x2c")
                nc.sync.dma_start(x2c[:], x2_dram[ci][:, sl])
                xbfc = bfc2_pool.tile([P, 512], bf16, tag="xbfc")
                nc.vector.tensor_copy(xbfc[:], x2c[:])
                sqc = bfc2_pool.tile([P, 512], bf16, tag="sqc")
                nc.vector.tensor_mul(sqc[:], x2c[:], x2c[:])
                nc.tensor.matmul(sx_ps[:], ones_bf[:], xbfc[:],
                                 start=(ci == 0), stop=(ci == NCT - 1))
                nc.tensor.matmul(sq_ps[:], ones_bf[:], sqc[:],
                                 start=(ci == 0), stop=(ci == NCT - 1))
            nc.scalar.mul(mu2[:, sl], sx_ps[:], 1.0 / C)
            nc.scalar.mul(ex22[:, sl], sq_ps[:], 1.0 / C)
        var2 = rows2_pool.tile([1, TL], f32, tag="row")
        nc.vector.tensor_mul(var2[:], mu2[:], mu2[:])
        nc.vector.scalar_tensor_tensor(var2[:], ex22[:], EPS,
                                       var2[:], OP.add, OP.subtract)
        std2 = rows2_pool.tile([1, TL], f32, tag="row")
        nc.scalar.activation(std2[:], var2[:], AF.Sqrt)
        rstd2 = rows2_pool.tile([1, TL], f32, tag="row")
        nc.vector.reciprocal(rstd2[:], std2[:])
        nm2 = rows2_pool.tile([1, TL], f32, tag="row")
        nc.scalar.mul(nm2[:], mu2[:], -1.0)

        xrow2_r = rowr_pool.tile([2, TL], bf16, tag="xrow2b")
        nc.gpsimd.dma_start(xrow2_r[0:1, :], nm2[:])
        nc.gpsimd.dma_start(xrow2_r[1:2, :], std2[:])

        for ch in range(2):
            sl = slice(ch * 512, (ch + 1) * 512)
            bc_ps = bcp_ps2.tile([P, 512], f32, tag="bc")
            nc.tensor.matmul(bc_ps[:], onescol[:], rstd2[:, sl],
                             start=True, stop=True)
            nc.scalar.copy(a2bc[ch][:], bc_ps[:])
        for ci in range(NCT):
            nc.gpsimd.dma_start(x2r[ci][:], x2_dram[ci])
        p5.close()

        # =========== Phase 6: FFN (LN folded, rstd2 deferred to ff2 evict) ===========
        p6 = ExitStack()
        w1_pool = p6.enter_context(tc.tile_pool(name="w1", bufs=16))
        ccf_pool = p6.enter_context(tc.tile_pool(name="ccf", bufs=4))
        w2_pool = p6.enter_context(tc.tile_pool(name="w2", bufs=16))
        ffn_ps = p6.enter_context(tc.tile_pool(name="ffnps", bufs=3, space="PSUM"))
        relu_pool = p6.enter_context(tc.tile_pool(name="relu", bufs=1))
        out_pool = p6.enter_context(tc.tile_pool(name="outsb", bufs=4))
        b2_pool = p6.enter_context(tc.tile_pool(name="b2p", bufs=1))

        b2col = [b2_pool.tile([P, 1], f32, name=f"b2c{ci}", tag=f"b2c{ci}")
                 for ci in range(NCT)]
        for ci in range(NCT):
            nc.sync.dma_start(b2col[ci][:], d_b2[ci].unsqueeze(1))

        relu1 = []
        for fog in range(NFT // 4):
            w1t = []
            for ci in range(NCT):
                w = w1_pool.tile([P, 512], bf16, tag="w1t")
                nc.sync.dma_start(w[:], d_w1[ci][:, fog * 512:(fog + 1) * 512])
                w1t.append(w)
            ccf = ccf_pool.tile([2, 512], bf16, tag="ccf")
            nc.sync.dma_start(ccf[:], d_ccf[fog])
            for fol in range(4):
                fo = fog * 4 + fol
                fsl = slice(fol * P, (fol + 1) * P)
                rt = relu_pool.tile([P, TL], bf16, name=f"rl{fo}", tag=f"rl{fo}")
                relu1.append(rt)
                for tc2 in range(2):
                    sl = slice(tc2 * 512, (tc2 + 1) * 512)
                    fp = ffn_ps.tile([P, 512], f32, tag="fp")
                    for ci in range(NCT):
                        nc.tensor.matmul(fp[:], w1t[ci][:, fsl], x2r[ci][:, sl],
                                         start=(ci == 0), stop=False)
                    nc.tensor.matmul(fp[:], ccf[:, fsl], xrow2_r[:, sl],
                                     start=False, stop=True)
                    nc.scalar.activation(rt[:, sl], fp[:], AF.Relu)

        for co in range(NCT):
            w2g = []
            for gq in range(NFT // 4):
                w = w2_pool.tile([P, 512], bf16, tag="w2t")
                nc.sync.dma_start(w[:], d_w2[co, gq])
                w2g.append(w)
            for tc2 in range(2):
                sl = slice(tc2 * 512, (tc2 + 1) * 512)
                fp = ffn_ps.tile([P, 512], f32, tag="fp")
                for fi in range(NFT):
                    nc.tensor.matmul(fp[:], w2g[fi // 4][:, (fi % 4) * P:(fi % 4 + 1) * P],
                                     relu1[fi][:, sl],
                                     start=(fi == 0), stop=(fi == NFT - 1))
                x2c = out_pool.tile([P, 512], f32, tag="x2c")
                nc.sync.dma_start(x2c[:], x2_dram[co][:, sl])
                tmp = out_pool.tile([P, 512], f32, tag="tmp")
                nc.vector.tensor_mul(tmp[:], fp[:], a2bc[tc2][:])
                osb = out_pool.tile([P, 512], f32, tag="osb")
                nc.vector.scalar_tensor_tensor(osb[:], tmp[:], b2col[co][:],
                                               x2c[:], OP.add, OP.add)
                nc.sync.dma_start(d_out[co][:, sl], osb[:])
        p6.close()

    nc.compile()
    return nc


def _prep_inputs(x, Wq, Wk, Wv, Wproj, bproj, W1, b1, W2, b2, g1, beta1, g2, beta2):
    """Build the 8 per-core input maps (host-side sharding + layout prep)."""
    f32 = np.float32
    scale = HS ** -0.5
    x = np.asarray(x, f32)
    Wq = np.asarray(Wq, f32)
    Wk = np.asarray(Wk, f32)
    Wv = np.asarray(Wv, f32)
    Wproj = np.asarray(Wproj, f32)
    W1 = np.asarray(W1, f32)
    b1 = np.asarray(b1, f32)
    W2 = np.asarray(W2, f32)
    g1 = np.asarray(g1, f32)
    beta1 = np.asarray(beta1, f32)
    g2 = np.asarray(g2, f32)
    beta2 = np.asarray(beta2, f32)

    w1g = g2[:, None] * W1
    b1p = b1 + beta2 @ W1
    w1bf = w1g.reshape(NCT, P, F).astype(ml_dtypes.bfloat16)
    ccfbf = np.stack([w1g.sum(0).reshape(NFT // 4, 512),
                      b1p.reshape(NFT // 4, 512)], axis=1).astype(ml_dtypes.bfloat16)
    # [NCT, NFT//4, P, 512]: w2h[co, gq, p, fi*128+cc] = W2[(gq*4+fi)*128+p, co*128+cc]
    w2bf = np.ascontiguousarray(
        W2.reshape(NFT // 4, 4, P, NCT, P).transpose(3, 0, 2, 1, 4).reshape(
            NCT, NFT // 4, P, 512)).astype(ml_dtypes.bfloat16)
    b2r = np.asarray(b2, f32).reshape(NCT, P)
    bprojr = np.asarray(bproj, f32).reshape(NCT, P)
    sp = np.arange(P)[:, None]
    qf = np.arange(512)[None, :]
    m01 = np.stack([(sp + 128 * off <= qf) for off in range(4)]).astype(
        ml_dtypes.bfloat16)
    ident = np.eye(P, dtype=f32)

    xT = [np.ascontiguousarray(x[b].T).reshape(NCT, P, T) for b in range(B)]

    def pair_weights(Wfull, g, scl):
        # raw pair weights [NPAIR, C, P]: cols 0:64 head g*8+2p, 64:128 head +1
        out = np.empty((NPAIR, C, P), f32)
        for p in range(NPAIR):
            hA, hB = g * 8 + 2 * p, g * 8 + 2 * p + 1
            out[p, :, 0:64] = Wfull[hA] * scl
            out[p, :, 64:128] = Wfull[hB] * scl
        return out

    per_g = {}
    for g in range(2):
        d = {}
        wqkv = np.empty((NPAIR, NCT, P, 3 * P), f32)
        ccqkv = np.empty((NPAIR, 2, 3 * P), f32)
        for j, (Wfull, scl) in enumerate(((Wq, scale), (Wk, 1.0), (Wv, 1.0))):
            raw = pair_weights(Wfull, g, scl)      # [NPAIR, C, P]
            wt = g1[None, :, None] * raw           # g1-folded
            wqkv[:, :, :, j * P:(j + 1) * P] = wt.reshape(NPAIR, NCT, P, P)
            ccqkv[:, 0, j * P:(j + 1) * P] = wt.sum(1)
            ccqkv[:, 1, j * P:(j + 1) * P] = np.einsum("c,pcd->pd", beta1, raw)
        d["wqkv"] = wqkv
        d["ccqkv"] = ccqkv
        d["wproj"] = np.ascontiguousarray(
            Wproj[g * 512:(g + 1) * 512]).reshape(NPAIR, P, C).astype(
                ml_dtypes.bfloat16)
        per_g[g] = d

    in_maps = []
    for c in range(8):
        b, g = c // 2, c % 2
        m = {
            "xT": xT[b],
            "xresT": np.ascontiguousarray(xT[b][:, :, g * TL:(g + 1) * TL]),
            "bproj": bprojr,
            "w1": w1bf,
            "ccf": ccfbf,
            "w2": w2bf,
            "b2": b2r,
            "m01": m01,
            "ident": ident,
        }
        m.update(per_g[g])
        in_maps.append(m)
    return in_maps


def kernel(**inputs):
    from concourse.bass_utils import run_bass_kernel_spmd

    if "nc" not in _CACHE:
        _CACHE["nc"] = _build(with_collective=True)
    nc = _CACHE["nc"]
    in_maps = _prep_inputs(**inputs)
    res = run_bass_kernel_spmd(nc, in_maps, list(range(8)))
    out = np.empty((B, T, C), np.float32)
    for c in range(8):
        b, g = c // 2, c % 2
        outT = res.results[c]["outT"].reshape(C, TL)
        out[b, g * TL:(g + 1) * TL, :] = outT.T
    return out


# revision 16
# speedup vs baseline: 1.7046x; 1.0776x over previous
"""Trainium2 Bass kernel for a dense transformer block (pre-LN, causal MHA + FFN).

Sharding (8 NeuronCores): core c = 2*b + g handles sequence b (of B=4) and
half g (of 2): tensor-parallel attention over 8 of 16 heads (partial proj,
pairwise ReduceScatter over {2b, 2b+1}), then token-parallel LN2+FFN over
its 1024 of 2048 tokens. Device kernel works in transposed [C, T] layout;
host transposes in/out.

LayerNorm is folded into the matmuls: for Q^T = Wq^T @ LN(x)^T we accumulate
M = W~^T x plus rank-1 corrections (colsum(W~) (x) -mu + (beta@W) (x) std)
in PSUM, then scale columns by rstd at eviction. Same trick for the FFN
(relu is positively homogeneous, so rstd2 commutes out to the ff2 evict).

Matmul dtypes: float32r (full-rate fp32 variant, fed via casting DMAs) for
QKV/S/FFN-1; bf16 for attention V*P, proj, and FFN-2.
"""
import numpy as np
import ml_dtypes
from contextlib import ExitStack

B, T, C = 4, 2048, 1024
H, HS = 16, 64
F = 4 * C
P = 128
EPS = 1e-5
NCT = C // P        # 8 c-tiles
NFT = F // P        # 32 f-tiles
TL = T // 2         # 1024 local tokens
NPAIR = 4           # head-pairs per core
GROUPS = [[0, 1], [2, 3], [4, 5], [6, 7]]

_CACHE = {}


def _build(with_collective=True):
    import concourse.bass as bass
    import concourse.tile as tile
    from concourse import bacc, mybir

    f32 = mybir.dt.float32
    f32r = mybir.dt.float32r
    bf16 = mybir.dt.bfloat16
    AF = mybir.ActivationFunctionType
    OP = mybir.AluOpType

    nc = bacc.Bacc("TRN2", target_bir_lowering=False, debug=False, num_devices=8)

    # ---- DRAM I/O ----
    d_xT = nc.dram_tensor("xT", [NCT, P, T], f32, kind="ExternalInput").ap()
    d_xres = nc.dram_tensor("xresT", [NCT, P, TL], f32, kind="ExternalInput").ap()
    d_wqkv = nc.dram_tensor("wqkv", [NPAIR, NCT, P, 3 * P], f32,
                            kind="ExternalInput").ap()
    d_ccqkv = nc.dram_tensor("ccqkv", [NPAIR, 2, 3 * P], f32,
                             kind="ExternalInput").ap()
    d_wproj = nc.dram_tensor("wproj", [NPAIR, P, C], bf16, kind="ExternalInput").ap()
    d_bproj = nc.dram_tensor("bproj", [NCT, P], f32, kind="ExternalInput").ap()
    d_w1 = nc.dram_tensor("w1", [NCT, P, F], bf16, kind="ExternalInput").ap()
    d_ccf = nc.dram_tensor("ccf", [NFT // 4, 2, 512], bf16, kind="ExternalInput").ap()
    d_w2 = nc.dram_tensor("w2", [NCT, NFT // 4, P, 512], bf16, kind="ExternalInput").ap()
    d_b2 = nc.dram_tensor("b2", [NCT, P], f32, kind="ExternalInput").ap()
    d_m01 = nc.dram_tensor("m01", [4, P, 512], bf16, kind="ExternalInput").ap()
    d_ident = nc.dram_tensor("ident", [P, P], f32, kind="ExternalInput").ap()
    d_out = nc.dram_tensor("outT", [NCT, P, TL], f32, kind="ExternalOutput").ap()

    with tile.TileContext(nc) as tc, ExitStack() as ctx:
        dram = ctx.enter_context(tc.tile_pool(name="dram", bufs=1, space="DRAM"))
        sa_bounce = dram.tile([2, NCT, P, TL], f32)
        sa_local = dram.tile([NCT, P, TL], f32)

        const = ctx.enter_context(tc.tile_pool(name="const", bufs=1))
        ones_bf = const.tile([P, 1], bf16)
        nc.vector.memset(ones_bf[:], 1.0)
        ident_f32 = const.tile([P, P], f32)
        nc.sync.dma_start(ident_f32[:], d_ident[:])
        masks = [const.tile([P, 512], bf16, name=f"mask{i}", tag=f"mask{i}")
                 for i in range(4)]
        for i in range(4):
            nc.sync.dma_start(masks[i][:], d_m01[i])
        ones1x64 = const.tile([1, 64], f32)
        nc.vector.memset(ones1x64[:], 1.0)
        onescol = const.tile([1, P], f32)
        nc.vector.memset(onescol[:], 1.0)

        x2_dram = dram.tile([NCT, P, TL], f32)

        # long-lived pools, first-use pinned bottom-up so frees are LIFO.
        abc_pool = ctx.enter_context(tc.tile_pool(name="abc", bufs=1))
        a1bc = [abc_pool.tile([P, 512], f32, name=f"a1bc{ch}", tag=f"a1bc{ch}")
                for ch in range(4)]
        a2bc = [abc_pool.tile([P, 512], f32, name=f"a2bc{ch}", tag=f"a2bc{ch}")
                for ch in range(2)]
        for t_ in a1bc + a2bc:
            nc.vector.memset(t_[:, 0:1], 0.0)  # pin allocation order
        rowr_pool = ctx.enter_context(tc.tile_pool(name="rowr", bufs=1))
        rowr_pin = rowr_pool.tile([1, 1], f32, tag="pin")
        nc.vector.memset(rowr_pin[:], 0.0)
        pattr = ExitStack()
        attT_pool = pattr.enter_context(tc.tile_pool(name="attT", bufs=1))
        attT = [attT_pool.tile([P, T], bf16, name=f"attT{p}", tag=f"attT{p}")
                for p in range(NPAIR)]
        for p in range(NPAIR):
            nc.vector.memset(attT[p][:, 0:1], 0.0)
        pqkv = ExitStack()
        vaug_pool = pqkv.enter_context(tc.tile_pool(name="vaug", bufs=1))
        qq_pool = pqkv.enter_context(tc.tile_pool(name="qq", bufs=1))
        kk_pool = pqkv.enter_context(tc.tile_pool(name="kk", bufs=1))
        v_aug = {}
        for p in range(NPAIR):
            for st in range(16):
                va = vaug_pool.tile([P, 130], bf16, name=f"va{p}_{st}",
                                    tag=f"va{p}_{st}")
                nc.vector.memset(va[:, 64:65], 1.0)
                nc.vector.memset(va[:, 129:130], 1.0)
                v_aug[(p, st)] = va
        qq_r = [qq_pool.tile([P, T], bf16, name=f"qq{p}", tag=f"qq{p}")
                for p in range(NPAIR)]
        kk_r = [kk_pool.tile([P, T], bf16, name=f"kk{p}", tag=f"kk{p}")
                for p in range(NPAIR)]
        for p in range(NPAIR):
            nc.vector.memset(qq_r[p][:, 0:1], 0.0)
            nc.vector.memset(kk_r[p][:, 0:1], 0.0)

        # =========== Phase 1: LN1 stats ===========
        p1 = ExitStack()
        xc_pool = p1.enter_context(tc.tile_pool(name="xc", bufs=4))
        bfc_pool = p1.enter_context(tc.tile_pool(name="bfc", bufs=6))
        rows1_pool = p1.enter_context(tc.tile_pool(name="rows1", bufs=4))
        stat_ps = p1.enter_context(tc.tile_pool(name="statps", bufs=2, space="PSUM"))
        bcp_ps = p1.enter_context(tc.tile_pool(name="bcpps", bufs=2, space="PSUM"))

        mu_row = rows1_pool.tile([1, T], f32, tag="row")
        ex2_row = rows1_pool.tile([1, T], f32, tag="row")
        for ch in range(4):
            sl = slice(ch * 512, (ch + 1) * 512)
            sx_ps = stat_ps.tile([1, 512], f32, tag="sx")
            sq_ps = stat_ps.tile([1, 512], f32, tag="sq")
            for ci in range(NCT):
                xc = xc_pool.tile([P, 512], f32, tag="xc")
                nc.sync.dma_start(xc[:], d_xT[ci][:, sl])
                xbfc = bfc_pool.tile([P, 512], bf16, tag="xbfc")
                nc.vector.tensor_copy(xbfc[:], xc[:])
                sqc = bfc_pool.tile([P, 512], bf16, tag="sqc")
                nc.vector.tensor_mul(sqc[:], xc[:], xc[:])
                nc.tensor.matmul(sx_ps[:], ones_bf[:], xbfc[:],
                                 start=(ci == 0), stop=(ci == NCT - 1))
                nc.tensor.matmul(sq_ps[:], ones_bf[:], sqc[:],
                                 start=(ci == 0), stop=(ci == NCT - 1))
            nc.scalar.mul(mu_row[:, sl], sx_ps[:], 1.0 / C)
            nc.scalar.mul(ex2_row[:, sl], sq_ps[:], 1.0 / C)

        var_row = rows1_pool.tile([1, T], f32, tag="row")
        nc.vector.tensor_mul(var_row[:], mu_row[:], mu_row[:])
        nc.vector.scalar_tensor_tensor(var_row[:], ex2_row[:], EPS,
                                       var_row[:], OP.add, OP.subtract)
        std_row = rows1_pool.tile([1, T], f32, tag="row")
        nc.scalar.activation(std_row[:], var_row[:], AF.Sqrt)
        rstd_row = rows1_pool.tile([1, T], f32, tag="row")
        nc.vector.reciprocal(rstd_row[:], std_row[:])
        nm_row = rows1_pool.tile([1, T], f32, tag="row")
        nc.scalar.mul(nm_row[:], mu_row[:], -1.0)

        xrow1_r = rowr_pool.tile([2, T], f32r, tag="xrowr")
        nc.gpsimd.dma_start(xrow1_r[0:1, :], nm_row[:])
        nc.gpsimd.dma_start(xrow1_r[1:2, :], std_row[:])

        for ch in range(4):
            sl = slice(ch * 512, (ch + 1) * 512)
            bc_ps = bcp_ps.tile([P, 512], f32, tag="bc")
            nc.tensor.matmul(bc_ps[:], onescol[:], rstd_row[:, sl],
                             start=True, stop=True)
            nc.scalar.copy(a1bc[ch][:], bc_ps[:])
        p1.close()

        # ===== Phases 2+3 (interleaved per pair): QKV + attention =====
        pat = ExitStack()
        w_pool = pat.enter_context(tc.tile_pool(name="wqkv", bufs=10))
        cc_pool = pat.enter_context(tc.tile_pool(name="cc", bufs=5))
        xr_pool = pat.enter_context(tc.tile_pool(name="xr", bufs=1))
        ev_pool = pat.enter_context(tc.tile_pool(name="ev", bufs=3))
        e_pool = pat.enter_context(tc.tile_pool(name="epool", bufs=6))
        rec_pool = pat.enter_context(tc.tile_pool(name="rec", bufs=2))
        bcsb_pool = pat.enter_context(tc.tile_pool(name="bcsb", bufs=2))
        mps = pat.enter_context(tc.tile_pool(name="mps", bufs=1, space="PSUM"))

        xr = [xr_pool.tile([P, T], f32r, name=f"xr{ci}", tag=f"xr{ci}")
              for ci in range(NCT)]
        for ci in range(NCT):
            nc.gpsimd.dma_start(xr[ci][:], d_xT[ci])

        for p in range(NPAIR):
            # --- QKV for pair p (LN folded via corrections + rstd evict scale) ---
            w_t = []
            for ci in range(NCT):
                w = w_pool.tile([P, 3 * P], f32r, tag="w")
                nc.gpsimd.dma_start(w[:], d_wqkv[p, ci])
                w_t.append(w)
            cc = cc_pool.tile([2, 3 * P], f32r, tag="cc")
            nc.gpsimd.dma_start(cc[:], d_ccqkv[p])
            for ch in range(4):
                sl = slice(ch * 512, (ch + 1) * 512)
                q_ps = mps.tile([P, 512], f32, tag="q_ps")
                k_ps = mps.tile([P, 512], f32, tag="k_ps")
                v_ps = mps.tile([P, 512], f32, tag="v_ps")
                for ci in range(NCT):
                    nc.tensor.matmul(q_ps[:], w_t[ci][:, 0:P], xr[ci][:, sl],
                                     start=(ci == 0), stop=False)
                    nc.tensor.matmul(k_ps[:], w_t[ci][:, P:2 * P], xr[ci][:, sl],
                                     start=(ci == 0), stop=False)
                    nc.tensor.matmul(v_ps[:], w_t[ci][:, 2 * P:3 * P], xr[ci][:, sl],
                                     start=(ci == 0), stop=False)
                nc.tensor.matmul(q_ps[:], cc[:, 0:P], xrow1_r[:, sl],
                                 start=False, stop=True)
                nc.tensor.matmul(k_ps[:], cc[:, P:2 * P], xrow1_r[:, sl],
                                 start=False, stop=True)
                nc.tensor.matmul(v_ps[:], cc[:, 2 * P:3 * P], xrow1_r[:, sl],
                                 start=False, stop=True)
                nc.vector.tensor_mul(qq_r[p][:, sl], q_ps[:], a1bc[ch][:])
                nc.vector.tensor_mul(kk_r[p][:, sl], k_ps[:], a1bc[ch][:])
                vev = ev_pool.tile([P, 512], f32, tag="vev")
                nc.vector.tensor_mul(vev[:], v_ps[:], a1bc[ch][:])
                for sti in range(4):
                    st = ch * 4 + sti
                    for hh in range(2):
                        tp = mps.tile([P, 64], f32, tag="tr")
                        nc.tensor.transpose(
                            tp[:], vev[hh * 64:(hh + 1) * 64, sti * 128:(sti + 1) * 128],
                            ident_f32[hh * 64:(hh + 1) * 64, hh * 64:(hh + 1) * 64])
                        nc.vector.tensor_copy(v_aug[(p, st)][:, hh * 65:hh * 65 + 64],
                                              tp[:])

            # --- attention for pair p (row-packed S^T, aug-row denominators) ---
            for qc in range(4):
                qsl = slice(qc * 512, (qc + 1) * 512)
                n_st = 4 * (qc + 1)
                attA = mps.tile([65, 512], f32, tag="attA")
                attB = mps.tile([65, 512], f32, tag="attB")
                for si in range(n_st):
                    ssl = slice(si * 128, (si + 1) * 128)
                    stA = mps.tile([P, 512], f32, tag="stA")
                    stB = mps.tile([P, 512], f32, tag="stB")
                    nc.tensor.matmul(stA[:], kk_r[p][0:64, ssl], qq_r[p][0:64, qsl],
                                     start=True, stop=True)
                    nc.tensor.matmul(stB[:], kk_r[p][64:128, ssl], qq_r[p][64:128, qsl],
                                     start=True, stop=True)
                    eA = e_pool.tile([P, 512], bf16, tag="e")
                    eB = e_pool.tile([P, 512], bf16, tag="e")
                    nc.scalar.activation(eA[:], stA[:], AF.Exp)
                    nc.scalar.activation(eB[:], stB[:], AF.Exp)
                    if si >= 4 * qc:
                        off = si - 4 * qc
                        nc.vector.tensor_mul(eA[:], eA[:], masks[off][:])
                        nc.vector.tensor_mul(eB[:], eB[:], masks[off][:])
                    nc.tensor.matmul(attA[:], v_aug[(p, si)][:, 0:65], eA[:],
                                     start=(si == 0), stop=(si == n_st - 1))
                    nc.tensor.matmul(attB[:], v_aug[(p, si)][:, 65:130], eB[:],
                                     start=(si == 0), stop=(si == n_st - 1))
                for hh, att in ((0, attA), (1, attB)):
                    rec = rec_pool.tile([1, 512], f32, tag="rec")
                    nc.vector.reciprocal(rec[:], att[64:65, :])
                    bc_sb = bcsb_pool.tile([64, 512], f32, tag="bc_sb")
                    nc.gpsimd.partition_broadcast(bc_sb[:], rec[:])
                    nc.vector.tensor_mul(attT[p][hh * 64:(hh + 1) * 64, qsl],
                                         att[0:64, :], bc_sb[:])
        pat.close()
        pqkv.close()

        # ===== Phase 4: proj -> ReduceScatter -> x2 (+ fused LN2 stats) =====
        px2bf = ExitStack()
        x2bf_pool = px2bf.enter_context(tc.tile_pool(name="x2bf", bufs=1))
        x2bf = [x2bf_pool.tile([P, TL], bf16, name=f"x2bf{ci}", tag=f"x2bf{ci}")
                for ci in range(NCT)]
        for ci in range(NCT):
            nc.vector.memset(x2bf[ci][:, 0:1], 0.0)  # pin allocation order

        p4 = ExitStack()
        wp_pool = p4.enter_context(tc.tile_pool(name="wproj", bufs=1))
        proj_ps = p4.enter_context(tc.tile_pool(name="projps", bufs=2, space="PSUM"))
        sa_pool = p4.enter_context(tc.tile_pool(name="sasb", bufs=4))
        xres_pool = p4.enter_context(tc.tile_pool(name="xres", bufs=2))
        bpj_pool = p4.enter_context(tc.tile_pool(name="bpj", bufs=1))
        sq_pool = p4.enter_context(tc.tile_pool(name="sq2", bufs=3))
        rows2_pool = p4.enter_context(tc.tile_pool(name="rows2", bufs=4))
        stat_ps2 = p4.enter_context(tc.tile_pool(name="statps2", bufs=2, space="PSUM"))
        bcp_ps2 = p4.enter_context(tc.tile_pool(name="bcpps2", bufs=1, space="PSUM"))

        bprojcol = [bpj_pool.tile([P, 1], f32, name=f"bpj{ci}", tag=f"bpj{ci}")
                    for ci in range(NCT)]
        for ci in range(NCT):
            nc.sync.dma_start(bprojcol[ci][:], d_bproj[ci].unsqueeze(1))
        wp = []
        for ki in range(NPAIR):
            w = wp_pool.tile([P, C], bf16, name=f"wp{ki}", tag=f"wp{ki}")
            nc.sync.dma_start(w[:], d_wproj[ki])
            wp.append(w)
        for co in range(NCT):
            for tc4 in range(4):
                sl = slice(tc4 * 512, (tc4 + 1) * 512)
                pp = proj_ps.tile([P, 512], f32, tag="pp")
                for ki in range(NPAIR):
                    nc.tensor.matmul(pp[:], wp[ki][:, co * P:(co + 1) * P],
                                     attT[ki][:, sl],
                                     start=(ki == 0), stop=(ki == NPAIR - 1))
                sa_sb = sa_pool.tile([P, 512], f32, tag="sa_sb")
                nc.vector.tensor_copy(sa_sb[:], pp[:])
                fold, off = tc4 // 2, (tc4 % 2) * 512
                nc.sync.dma_start(sa_bounce[fold, co, :, off:off + 512], sa_sb[:])

        if with_collective:
            nc.gpsimd.collective_compute(
                "ReduceScatter",
                OP.add,
                replica_groups=GROUPS,
                ins=[sa_bounce.opt()],
                outs=[sa_local.opt()],
            )
        else:
            nc.sync.dma_start(sa_local[:], sa_bounce[0])

        sx_ch = [stat_ps2.tile([1, 512], f32, name=f"sx2_{ch}", tag="sx")
                 for ch in range(2)]
        sq_ch = [stat_ps2.tile([1, 512], f32, name=f"sq2_{ch}", tag="sq")
                 for ch in range(2)]
        for co in range(NCT):
            sal = xres_pool.tile([P, TL], f32, tag="sal")
            nc.sync.dma_start(sal[:], sa_local[co])
            xres = xres_pool.tile([P, TL], f32, tag="xres")
            nc.sync.dma_start(xres[:], d_xres[co])
            x2sb = xres_pool.tile([P, TL], f32, tag="x2sb")
            nc.vector.scalar_tensor_tensor(x2sb[:], sal[:], bprojcol[co][:],
                                           xres[:], OP.add, OP.add)
            nc.sync.dma_start(x2_dram[co], x2sb[:])
            nc.vector.tensor_copy(x2bf[co][:], x2sb[:])
            sqt = sq_pool.tile([P, TL], bf16, tag="sqt")
            nc.vector.tensor_mul(sqt[:], x2sb[:], x2sb[:])
            for ch in range(2):
                sl = slice(ch * 512, (ch + 1) * 512)
                nc.tensor.matmul(sx_ch[ch][:], ones_bf[:], x2bf[co][:, sl],
                                 start=(co == 0), stop=(co == NCT - 1))
                nc.tensor.matmul(sq_ch[ch][:], ones_bf[:], sqt[:, sl],
                                 start=(co == 0), stop=(co == NCT - 1))

        mu2 = rows2_pool.tile([1, TL], f32, tag="row")
        ex22 = rows2_pool.tile([1, TL], f32, tag="row")
        for ch in range(2):
            sl = slice(ch * 512, (ch + 1) * 512)
            nc.scalar.mul(mu2[:, sl], sx_ch[ch][:], 1.0 / C)
            nc.scalar.mul(ex22[:, sl], sq_ch[ch][:], 1.0 / C)
        var2 = rows2_pool.tile([1, TL], f32, tag="row")
        nc.vector.tensor_mul(var2[:], mu2[:], mu2[:])
        nc.vector.scalar_tensor_tensor(var2[:], ex22[:], EPS,
                                       var2[:], OP.add, OP.subtract)
        std2 = rows2_pool.tile([1, TL], f32, tag="row")
        nc.scalar.activation(std2[:], var2[:], AF.Sqrt)
        rstd2 = rows2_pool.tile([1, TL], f32, tag="row")
        nc.vector.reciprocal(rstd2[:], std2[:])
        nm2 = rows2_pool.tile([1, TL], f32, tag="row")
        nc.scalar.mul(nm2[:], mu2[:], -1.0)

        xrow2_r = rowr_pool.tile([2, TL], bf16, tag="xrow2b")
        nc.gpsimd.dma_start(xrow2_r[0:1, :], nm2[:])
        nc.gpsimd.dma_start(xrow2_r[1:2, :], std2[:])
        for ch in range(2):
            sl = slice(ch * 512, (ch + 1) * 512)
            bc_ps = bcp_ps2.tile([P, 512], f32, tag="bc")
            nc.tensor.matmul(bc_ps[:], onescol[:], rstd2[:, sl],
                             start=True, stop=True)
            nc.scalar.copy(a2bc[ch][:], bc_ps[:])
        p4.close()

        # =========== Phase 6: FFN (LN folded, rstd2 deferred to ff2 evict) ===========
        p6 = ExitStack()
        w1_pool = p6.enter_context(tc.tile_pool(name="w1", bufs=16))
        ccf_pool = p6.enter_context(tc.tile_pool(name="ccf", bufs=4))
        w2_pool = p6.enter_context(tc.tile_pool(name="w2", bufs=16))
        ffn_ps = p6.enter_context(tc.tile_pool(name="ffnps", bufs=3, space="PSUM"))
        relu_pool = p6.enter_context(tc.tile_pool(name="relu", bufs=1))
        out_pool = p6.enter_context(tc.tile_pool(name="outsb", bufs=4))
        b2_pool = p6.enter_context(tc.tile_pool(name="b2p", bufs=1))

        b2col = [b2_pool.tile([P, 1], f32, name=f"b2c{ci}", tag=f"b2c{ci}")
                 for ci in range(NCT)]
        for ci in range(NCT):
            nc.sync.dma_start(b2col[ci][:], d_b2[ci].unsqueeze(1))

        relu1 = []
        for fog in range(NFT // 4):
            w1t = []
            for ci in range(NCT):
                w = w1_pool.tile([P, 512], bf16, tag="w1t")
                nc.sync.dma_start(w[:], d_w1[ci][:, fog * 512:(fog + 1) * 512])
                w1t.append(w)
            ccf = ccf_pool.tile([2, 512], bf16, tag="ccf")
            nc.sync.dma_start(ccf[:], d_ccf[fog])
            for fol in range(4):
                fo = fog * 4 + fol
                fsl = slice(fol * P, (fol + 1) * P)
                rt = relu_pool.tile([P, TL], bf16, name=f"rl{fo}", tag=f"rl{fo}")
                relu1.append(rt)
                for tc2 in range(2):
                    sl = slice(tc2 * 512, (tc2 + 1) * 512)
                    fp = ffn_ps.tile([P, 512], f32, tag="fp")
                    for ci in range(NCT):
                        nc.tensor.matmul(fp[:], w1t[ci][:, fsl], x2bf[ci][:, sl],
                                         start=(ci == 0), stop=False)
                    nc.tensor.matmul(fp[:], ccf[:, fsl], xrow2_r[:, sl],
                                     start=False, stop=True)
                    nc.scalar.activation(rt[:, sl], fp[:], AF.Relu)

        for co in range(NCT):
            w2g = []
            for gq in range(NFT // 4):
                w = w2_pool.tile([P, 512], bf16, tag="w2t")
                nc.sync.dma_start(w[:], d_w2[co, gq])
                w2g.append(w)
            for tc2 in range(2):
                sl = slice(tc2 * 512, (tc2 + 1) * 512)
                fp = ffn_ps.tile([P, 512], f32, tag="fp")
                for fi in range(NFT):
                    nc.tensor.matmul(fp[:], w2g[fi // 4][:, (fi % 4) * P:(fi % 4 + 1) * P],
                                     relu1[fi][:, sl],
                                     start=(fi == 0), stop=(fi == NFT - 1))
                x2c = out_pool.tile([P, 512], f32, tag="x2c")
                nc.sync.dma_start(x2c[:], x2_dram[co][:, sl])
                tmp = out_pool.tile([P, 512], f32, tag="tmp")
                nc.vector.tensor_mul(tmp[:], fp[:], a2bc[tc2][:])
                osb = out_pool.tile([P, 512], f32, tag="osb")
                nc.vector.scalar_tensor_tensor(osb[:], tmp[:], b2col[co][:],
                                               x2c[:], OP.add, OP.add)
                nc.sync.dma_start(d_out[co][:, sl], osb[:])
        p6.close()
        px2bf.close()
        pattr.close()

    nc.compile()
    return nc


def _prep_inputs(x, Wq, Wk, Wv, Wproj, bproj, W1, b1, W2, b2, g1, beta1, g2, beta2):
    """Build the 8 per-core input maps (host-side sharding + layout prep)."""
    f32 = np.float32
    scale = HS ** -0.5
    x = np.asarray(x, f32)
    Wq = np.asarray(Wq, f32)
    Wk = np.asarray(Wk, f32)
    Wv = np.asarray(Wv, f32)
    Wproj = np.asarray(Wproj, f32)
    W1 = np.asarray(W1, f32)
    b1 = np.asarray(b1, f32)
    W2 = np.asarray(W2, f32)
    g1 = np.asarray(g1, f32)
    beta1 = np.asarray(beta1, f32)
    g2 = np.asarray(g2, f32)
    beta2 = np.asarray(beta2, f32)

    w1g = g2[:, None] * W1
    b1p = b1 + beta2 @ W1
    w1bf = w1g.reshape(NCT, P, F).astype(ml_dtypes.bfloat16)
    ccfbf = np.stack([w1g.sum(0).reshape(NFT // 4, 512),
                      b1p.reshape(NFT // 4, 512)], axis=1).astype(ml_dtypes.bfloat16)
    # [NCT, NFT//4, P, 512]: w2h[co, gq, p, fi*128+cc] = W2[(gq*4+fi)*128+p, co*128+cc]
    w2bf = np.ascontiguousarray(
        W2.reshape(NFT // 4, 4, P, NCT, P).transpose(3, 0, 2, 1, 4).reshape(
            NCT, NFT // 4, P, 512)).astype(ml_dtypes.bfloat16)
    b2r = np.asarray(b2, f32).reshape(NCT, P)
    bprojr = np.asarray(bproj, f32).reshape(NCT, P)
    sp = np.arange(P)[:, None]
    qf = np.arange(512)[None, :]
    m01 = np.stack([(sp + 128 * off <= qf) for off in range(4)]).astype(
        ml_dtypes.bfloat16)
    ident = np.eye(P, dtype=f32)

    xT = [np.ascontiguousarray(x[b].T).reshape(NCT, P, T) for b in range(B)]

    def pair_weights(Wfull, g, scl):
        # raw pair weights [NPAIR, C, P]: cols 0:64 head g*8+2p, 64:128 head +1
        out = np.empty((NPAIR, C, P), f32)
        for p in range(NPAIR):
            hA, hB = g * 8 + 2 * p, g * 8 + 2 * p + 1
            out[p, :, 0:64] = Wfull[hA] * scl
            out[p, :, 64:128] = Wfull[hB] * scl
        return out

    per_g = {}
    for g in range(2):
        d = {}
        wqkv = np.empty((NPAIR, NCT, P, 3 * P), f32)
        ccqkv = np.empty((NPAIR, 2, 3 * P), f32)
        for j, (Wfull, scl) in enumerate(((Wq, scale), (Wk, 1.0), (Wv, 1.0))):
            raw = pair_weights(Wfull, g, scl)      # [NPAIR, C, P]
            wt = g1[None, :, None] * raw           # g1-folded
            wqkv[:, :, :, j * P:(j + 1) * P] = wt.reshape(NPAIR, NCT, P, P)
            ccqkv[:, 0, j * P:(j + 1) * P] = wt.sum(1)
            ccqkv[:, 1, j * P:(j + 1) * P] = np.einsum("c,pcd->pd", beta1, raw)
        d["wqkv"] = wqkv
        d["ccqkv"] = ccqkv
        d["wproj"] = np.ascontiguousarray(
            Wproj[g * 512:(g + 1) * 512]).reshape(NPAIR, P, C).astype(
                ml_dtypes.bfloat16)
        per_g[g] = d

    in_maps = []
    for c in range(8):
        b, g = c // 2, c % 2
        m = {
            "xT": xT[b],
            "xresT": np.ascontiguousarray(xT[b][:, :, g * TL:(g + 1) * TL]),
            "bproj": bprojr,
            "w1": w1bf,
            "ccf": ccfbf,
            "w2": w2bf,
            "b2": b2r,
            "m01": m01,
            "ident": ident,
        }
        m.update(per_g[g])
        in_maps.append(m)
    return in_maps


def kernel(**inputs):
    from concourse.bass_utils import run_bass_kernel_spmd

    if "nc" not in _CACHE:
        _CACHE["nc"] = _build(with_collective=True)
    nc = _CACHE["nc"]
    in_maps = _prep_inputs(**inputs)
    res = run_bass_kernel_spmd(nc, in_maps, list(range(8)))
    out = np.empty((B, T, C), np.float32)
    for c in range(8):
        b, g = c // 2, c % 2
        outT = res.results[c]["outT"].reshape(C, TL)
        out[b, g * TL:(g + 1) * TL, :] = outT.T
    return out


# revision 17
# speedup vs baseline: 1.7176x; 1.0076x over previous
"""Trainium2 Bass kernel for a dense transformer block (pre-LN, causal MHA + FFN).

Sharding (8 NeuronCores): core c = 2*b + g handles sequence b (of B=4) and
half g (of 2): tensor-parallel attention over 8 of 16 heads (partial proj,
pairwise ReduceScatter over {2b, 2b+1}), then token-parallel LN2+FFN over
its 1024 of 2048 tokens. Device kernel works in transposed [C, T] layout;
host transposes in/out.

LayerNorm is folded into the matmuls: for Q^T = Wq^T @ LN(x)^T we accumulate
M = W~^T x plus rank-1 corrections (colsum(W~) (x) -mu + (beta@W) (x) std)
in PSUM, then scale columns by rstd at eviction. Same trick for the FFN
(relu is positively homogeneous, so rstd2 commutes out to the ff2 evict).

Matmul dtypes: float32r (full-rate fp32 variant, fed via casting DMAs) for
QKV/S/FFN-1; bf16 for attention V*P, proj, and FFN-2.
"""
import numpy as np
import ml_dtypes
from contextlib import ExitStack

B, T, C = 4, 2048, 1024
H, HS = 16, 64
F = 4 * C
P = 128
EPS = 1e-5
NCT = C // P        # 8 c-tiles
NFT = F // P        # 32 f-tiles
TL = T // 2         # 1024 local tokens
NPAIR = 4           # head-pairs per core
GROUPS = [[0, 1], [2, 3], [4, 5], [6, 7]]

_CACHE = {}


def _build(with_collective=True):
    import concourse.bass as bass
    import concourse.tile as tile
    from concourse import bacc, mybir

    f32 = mybir.dt.float32
    f32r = mybir.dt.float32r
    bf16 = mybir.dt.bfloat16
    AF = mybir.ActivationFunctionType
    OP = mybir.AluOpType

    nc = bacc.Bacc("TRN2", target_bir_lowering=False, debug=False, num_devices=8)

    # ---- DRAM I/O ----
    d_xT = nc.dram_tensor("xT", [NCT, P, T], f32, kind="ExternalInput").ap()
    d_xres = nc.dram_tensor("xresT", [NCT, P, TL], f32, kind="ExternalInput").ap()
    d_wqkv = nc.dram_tensor("wqkv", [NPAIR, NCT, P, 3 * P], f32,
                            kind="ExternalInput").ap()
    d_ccqkv = nc.dram_tensor("ccqkv", [NPAIR, 2, 3 * P], f32,
                             kind="ExternalInput").ap()
    d_wproj = nc.dram_tensor("wproj", [NPAIR, P, C], bf16, kind="ExternalInput").ap()
    d_bproj = nc.dram_tensor("bproj", [NCT, P], f32, kind="ExternalInput").ap()
    d_w1 = nc.dram_tensor("w1", [NCT, P, F], bf16, kind="ExternalInput").ap()
    d_ccf = nc.dram_tensor("ccf", [NFT // 4, 2, 512], bf16, kind="ExternalInput").ap()
    d_w2 = nc.dram_tensor("w2", [NCT, NFT // 4, P, 512], bf16, kind="ExternalInput").ap()
    d_b2 = nc.dram_tensor("b2", [NCT, P], f32, kind="ExternalInput").ap()
    d_m01 = nc.dram_tensor("m01", [4, P, 512], bf16, kind="ExternalInput").ap()
    d_ident = nc.dram_tensor("ident", [P, P], f32, kind="ExternalInput").ap()
    d_out = nc.dram_tensor("outT", [NCT, P, TL], f32, kind="ExternalOutput").ap()

    with tile.TileContext(nc) as tc, ExitStack() as ctx:
        dram = ctx.enter_context(tc.tile_pool(name="dram", bufs=1, space="DRAM"))
        sa_bounce = [dram.tile([2, NCT // 2, P, TL], f32, name=f"sab{h}")
                     for h in range(2)]
        sa_local = [dram.tile([NCT // 2, P, TL], f32, name=f"sal{h}")
                    for h in range(2)]

        const = ctx.enter_context(tc.tile_pool(name="const", bufs=1))
        ones_bf = const.tile([P, 1], bf16)
        nc.vector.memset(ones_bf[:], 1.0)
        ident_f32 = const.tile([P, P], f32)
        nc.sync.dma_start(ident_f32[:], d_ident[:])
        masks = [const.tile([P, 512], bf16, name=f"mask{i}", tag=f"mask{i}")
                 for i in range(4)]
        for i in range(4):
            nc.sync.dma_start(masks[i][:], d_m01[i])
        ones1x64 = const.tile([1, 64], f32)
        nc.vector.memset(ones1x64[:], 1.0)
        onescol = const.tile([1, P], f32)
        nc.vector.memset(onescol[:], 1.0)

        x2_dram = dram.tile([NCT, P, TL], f32)

        # long-lived pools, first-use pinned bottom-up so frees are LIFO.
        abc_pool = ctx.enter_context(tc.tile_pool(name="abc", bufs=1))
        a1bc = [abc_pool.tile([P, 512], f32, name=f"a1bc{ch}", tag=f"a1bc{ch}")
                for ch in range(4)]
        a2bc = [abc_pool.tile([P, 512], f32, name=f"a2bc{ch}", tag=f"a2bc{ch}")
                for ch in range(2)]
        for t_ in a1bc + a2bc:
            nc.vector.memset(t_[:, 0:1], 0.0)  # pin allocation order
        rowr_pool = ctx.enter_context(tc.tile_pool(name="rowr", bufs=1))
        rowr_pin = rowr_pool.tile([1, 1], f32, tag="pin")
        nc.vector.memset(rowr_pin[:], 0.0)
        pattr = ExitStack()
        attT_pool = pattr.enter_context(tc.tile_pool(name="attT", bufs=1))
        attT = [attT_pool.tile([P, T], bf16, name=f"attT{p}", tag=f"attT{p}")
                for p in range(NPAIR)]
        for p in range(NPAIR):
            nc.vector.memset(attT[p][:, 0:1], 0.0)
        pqkv = ExitStack()
        vaug_pool = pqkv.enter_context(tc.tile_pool(name="vaug", bufs=1))
        qq_pool = pqkv.enter_context(tc.tile_pool(name="qq", bufs=1))
        kk_pool = pqkv.enter_context(tc.tile_pool(name="kk", bufs=1))
        v_aug = {}
        for p in range(NPAIR):
            for st in range(16):
                va = vaug_pool.tile([P, 130], bf16, name=f"va{p}_{st}",
                                    tag=f"va{p}_{st}")
                nc.vector.memset(va[:, 64:65], 1.0)
                nc.vector.memset(va[:, 129:130], 1.0)
                v_aug[(p, st)] = va
        qq_r = [qq_pool.tile([P, T], bf16, name=f"qq{p}", tag=f"qq{p}")
                for p in range(NPAIR)]
        kk_r = [kk_pool.tile([P, T], bf16, name=f"kk{p}", tag=f"kk{p}")
                for p in range(NPAIR)]
        for p in range(NPAIR):
            nc.vector.memset(qq_r[p][:, 0:1], 0.0)
            nc.vector.memset(kk_r[p][:, 0:1], 0.0)

        # =========== Phase 1: LN1 stats ===========
        p1 = ExitStack()
        xc_pool = p1.enter_context(tc.tile_pool(name="xc", bufs=4))
        bfc_pool = p1.enter_context(tc.tile_pool(name="bfc", bufs=6))
        rows1_pool = p1.enter_context(tc.tile_pool(name="rows1", bufs=4))
        stat_ps = p1.enter_context(tc.tile_pool(name="statps", bufs=2, space="PSUM"))
        bcp_ps = p1.enter_context(tc.tile_pool(name="bcpps", bufs=2, space="PSUM"))

        mu_row = rows1_pool.tile([1, T], f32, tag="row")
        ex2_row = rows1_pool.tile([1, T], f32, tag="row")
        for ch in range(4):
            sl = slice(ch * 512, (ch + 1) * 512)
            sx_ps = stat_ps.tile([1, 512], f32, tag="sx")
            sq_ps = stat_ps.tile([1, 512], f32, tag="sq")
            for ci in range(NCT):
                xc = xc_pool.tile([P, 512], f32, tag="xc")
                nc.sync.dma_start(xc[:], d_xT[ci][:, sl])
                xbfc = bfc_pool.tile([P, 512], bf16, tag="xbfc")
                nc.vector.tensor_copy(xbfc[:], xc[:])
                sqc = bfc_pool.tile([P, 512], bf16, tag="sqc")
                nc.vector.tensor_mul(sqc[:], xc[:], xc[:])
                nc.tensor.matmul(sx_ps[:], ones_bf[:], xbfc[:],
                                 start=(ci == 0), stop=(ci == NCT - 1))
                nc.tensor.matmul(sq_ps[:], ones_bf[:], sqc[:],
                                 start=(ci == 0), stop=(ci == NCT - 1))
            nc.scalar.mul(mu_row[:, sl], sx_ps[:], 1.0 / C)
            nc.scalar.mul(ex2_row[:, sl], sq_ps[:], 1.0 / C)

        var_row = rows1_pool.tile([1, T], f32, tag="row")
        nc.vector.tensor_mul(var_row[:], mu_row[:], mu_row[:])
        nc.vector.scalar_tensor_tensor(var_row[:], ex2_row[:], EPS,
                                       var_row[:], OP.add, OP.subtract)
        std_row = rows1_pool.tile([1, T], f32, tag="row")
        nc.scalar.activation(std_row[:], var_row[:], AF.Sqrt)
        rstd_row = rows1_pool.tile([1, T], f32, tag="row")
        nc.vector.reciprocal(rstd_row[:], std_row[:])
        nm_row = rows1_pool.tile([1, T], f32, tag="row")
        nc.scalar.mul(nm_row[:], mu_row[:], -1.0)

        xrow1_r = rowr_pool.tile([2, T], f32r, tag="xrowr")
        nc.gpsimd.dma_start(xrow1_r[0:1, :], nm_row[:])
        nc.gpsimd.dma_start(xrow1_r[1:2, :], std_row[:])

        for ch in range(4):
            sl = slice(ch * 512, (ch + 1) * 512)
            bc_ps = bcp_ps.tile([P, 512], f32, tag="bc")
            nc.tensor.matmul(bc_ps[:], onescol[:], rstd_row[:, sl],
                             start=True, stop=True)
            nc.scalar.copy(a1bc[ch][:], bc_ps[:])
        p1.close()

        # ===== Phases 2+3 (interleaved per pair): QKV + attention =====
        pat = ExitStack()
        w_pool = pat.enter_context(tc.tile_pool(name="wqkv", bufs=10))
        cc_pool = pat.enter_context(tc.tile_pool(name="cc", bufs=5))
        xr_pool = pat.enter_context(tc.tile_pool(name="xr", bufs=1))
        ev_pool = pat.enter_context(tc.tile_pool(name="ev", bufs=3))
        e_pool = pat.enter_context(tc.tile_pool(name="epool", bufs=6))
        rec_pool = pat.enter_context(tc.tile_pool(name="rec", bufs=2))
        bcsb_pool = pat.enter_context(tc.tile_pool(name="bcsb", bufs=2))
        mps = pat.enter_context(tc.tile_pool(name="mps", bufs=1, space="PSUM"))

        xr = [xr_pool.tile([P, T], f32r, name=f"xr{ci}", tag=f"xr{ci}")
              for ci in range(NCT)]
        for ci in range(NCT):
            nc.gpsimd.dma_start(xr[ci][:], d_xT[ci])

        for p in range(NPAIR):
            # --- QKV for pair p (LN folded via corrections + rstd evict scale) ---
            w_t = []
            for ci in range(NCT):
                w = w_pool.tile([P, 3 * P], f32r, tag="w")
                nc.gpsimd.dma_start(w[:], d_wqkv[p, ci])
                w_t.append(w)
            cc = cc_pool.tile([2, 3 * P], f32r, tag="cc")
            nc.gpsimd.dma_start(cc[:], d_ccqkv[p])
            for ch in range(4):
                sl = slice(ch * 512, (ch + 1) * 512)
                q_ps = mps.tile([P, 512], f32, tag="q_ps")
                k_ps = mps.tile([P, 512], f32, tag="k_ps")
                v_ps = mps.tile([P, 512], f32, tag="v_ps")
                for ci in range(NCT):
                    nc.tensor.matmul(q_ps[:], w_t[ci][:, 0:P], xr[ci][:, sl],
                                     start=(ci == 0), stop=False)
                    nc.tensor.matmul(k_ps[:], w_t[ci][:, P:2 * P], xr[ci][:, sl],
                                     start=(ci == 0), stop=False)
                    nc.tensor.matmul(v_ps[:], w_t[ci][:, 2 * P:3 * P], xr[ci][:, sl],
                                     start=(ci == 0), stop=False)
                nc.tensor.matmul(q_ps[:], cc[:, 0:P], xrow1_r[:, sl],
                                 start=False, stop=True)
                nc.tensor.matmul(k_ps[:], cc[:, P:2 * P], xrow1_r[:, sl],
                                 start=False, stop=True)
                nc.tensor.matmul(v_ps[:], cc[:, 2 * P:3 * P], xrow1_r[:, sl],
                                 start=False, stop=True)
                nc.vector.tensor_mul(qq_r[p][:, sl], q_ps[:], a1bc[ch][:])
                nc.vector.tensor_mul(kk_r[p][:, sl], k_ps[:], a1bc[ch][:])
                vev = ev_pool.tile([P, 512], f32, tag="vev")
                nc.vector.tensor_mul(vev[:], v_ps[:], a1bc[ch][:])
                for sti in range(4):
                    st = ch * 4 + sti
                    for hh in range(2):
                        tp = mps.tile([P, 64], f32, tag="tr")
                        nc.tensor.transpose(
                            tp[:], vev[hh * 64:(hh + 1) * 64, sti * 128:(sti + 1) * 128],
                            ident_f32[hh * 64:(hh + 1) * 64, hh * 64:(hh + 1) * 64])
                        nc.vector.tensor_copy(v_aug[(p, st)][:, hh * 65:hh * 65 + 64],
                                              tp[:])

            # --- attention for pair p (row-packed S^T, aug-row denominators) ---
            for qc in range(4):
                qsl = slice(qc * 512, (qc + 1) * 512)
                n_st = 4 * (qc + 1)
                attA = mps.tile([65, 512], f32, tag="attA")
                attB = mps.tile([65, 512], f32, tag="attB")
                for si in range(n_st):
                    ssl = slice(si * 128, (si + 1) * 128)
                    stA = mps.tile([P, 512], f32, tag="stA")
                    stB = mps.tile([P, 512], f32, tag="stB")
                    nc.tensor.matmul(stA[:], kk_r[p][0:64, ssl], qq_r[p][0:64, qsl],
                                     start=True, stop=True)
                    nc.tensor.matmul(stB[:], kk_r[p][64:128, ssl], qq_r[p][64:128, qsl],
                                     start=True, stop=True)
                    eA = e_pool.tile([P, 512], bf16, tag="e")
                    eB = e_pool.tile([P, 512], bf16, tag="e")
                    nc.scalar.activation(eA[:], stA[:], AF.Exp)
                    nc.scalar.activation(eB[:], stB[:], AF.Exp)
                    if si >= 4 * qc:
                        off = si - 4 * qc
                        nc.vector.tensor_mul(eA[:], eA[:], masks[off][:])
                        nc.vector.tensor_mul(eB[:], eB[:], masks[off][:])
                    nc.tensor.matmul(attA[:], v_aug[(p, si)][:, 0:65], eA[:],
                                     start=(si == 0), stop=(si == n_st - 1))
                    nc.tensor.matmul(attB[:], v_aug[(p, si)][:, 65:130], eB[:],
                                     start=(si == 0), stop=(si == n_st - 1))
                for hh, att in ((0, attA), (1, attB)):
                    rec = rec_pool.tile([1, 512], f32, tag="rec")
                    nc.vector.reciprocal(rec[:], att[64:65, :])
                    bc_sb = bcsb_pool.tile([64, 512], f32, tag="bc_sb")
                    nc.gpsimd.partition_broadcast(bc_sb[:], rec[:])
                    nc.vector.tensor_mul(attT[p][hh * 64:(hh + 1) * 64, qsl],
                                         att[0:64, :], bc_sb[:])
        pat.close()
        pqkv.close()

        # ===== Phase 4: proj -> ReduceScatter -> x2 (+ fused LN2 stats) =====
        px2bf = ExitStack()
        x2bf_pool = px2bf.enter_context(tc.tile_pool(name="x2bf", bufs=1))
        x2bf = [x2bf_pool.tile([P, TL], bf16, name=f"x2bf{ci}", tag=f"x2bf{ci}")
                for ci in range(NCT)]
        for ci in range(NCT):
            nc.vector.memset(x2bf[ci][:, 0:1], 0.0)  # pin allocation order

        p4 = ExitStack()
        wp_pool = p4.enter_context(tc.tile_pool(name="wproj", bufs=1))
        proj_ps = p4.enter_context(tc.tile_pool(name="projps", bufs=2, space="PSUM"))
        sa_pool = p4.enter_context(tc.tile_pool(name="sasb", bufs=4))
        xres_pool = p4.enter_context(tc.tile_pool(name="xres", bufs=2))
        bpj_pool = p4.enter_context(tc.tile_pool(name="bpj", bufs=1))
        sq_pool = p4.enter_context(tc.tile_pool(name="sq2", bufs=3))
        rows2_pool = p4.enter_context(tc.tile_pool(name="rows2", bufs=4))
        stat_ps2 = p4.enter_context(tc.tile_pool(name="statps2", bufs=2, space="PSUM"))
        bcp_ps2 = p4.enter_context(tc.tile_pool(name="bcpps2", bufs=1, space="PSUM"))

        bprojcol = [bpj_pool.tile([P, 1], f32, name=f"bpj{ci}", tag=f"bpj{ci}")
                    for ci in range(NCT)]
        for ci in range(NCT):
            nc.sync.dma_start(bprojcol[ci][:], d_bproj[ci].unsqueeze(1))
        wp = []
        for ki in range(NPAIR):
            w = wp_pool.tile([P, C], bf16, name=f"wp{ki}", tag=f"wp{ki}")
            nc.sync.dma_start(w[:], d_wproj[ki])
            wp.append(w)
        for co in range(NCT):
            for tc4 in range(4):
                sl = slice(tc4 * 512, (tc4 + 1) * 512)
                pp = proj_ps.tile([P, 512], f32, tag="pp")
                for ki in range(NPAIR):
                    nc.tensor.matmul(pp[:], wp[ki][:, co * P:(co + 1) * P],
                                     attT[ki][:, sl],
                                     start=(ki == 0), stop=(ki == NPAIR - 1))
                sa_sb = sa_pool.tile([P, 512], f32, tag="sa_sb")
                nc.vector.tensor_copy(sa_sb[:], pp[:])
                fold, off = tc4 // 2, (tc4 % 2) * 512
                nc.sync.dma_start(
                    sa_bounce[co // 4][fold, co % 4, :, off:off + 512], sa_sb[:])
            if co == 3 or co == NCT - 1:
                h = co // 4
                if with_collective:
                    nc.gpsimd.collective_compute(
                        "ReduceScatter",
                        OP.add,
                        replica_groups=GROUPS,
                        ins=[sa_bounce[h].opt()],
                        outs=[sa_local[h].opt()],
                    )
                else:
                    nc.sync.dma_start(sa_local[h][:], sa_bounce[h][0])

        sx_ch = [stat_ps2.tile([1, 512], f32, name=f"sx2_{ch}", tag="sx")
                 for ch in range(2)]
        sq_ch = [stat_ps2.tile([1, 512], f32, name=f"sq2_{ch}", tag="sq")
                 for ch in range(2)]
        for co in range(NCT):
            sal = xres_pool.tile([P, TL], f32, tag="sal")
            nc.sync.dma_start(sal[:], sa_local[co // 4][co % 4])
            xres = xres_pool.tile([P, TL], f32, tag="xres")
            nc.sync.dma_start(xres[:], d_xres[co])
            x2sb = xres_pool.tile([P, TL], f32, tag="x2sb")
            nc.vector.scalar_tensor_tensor(x2sb[:], sal[:], bprojcol[co][:],
                                           xres[:], OP.add, OP.add)
            nc.sync.dma_start(x2_dram[co], x2sb[:])
            nc.vector.tensor_copy(x2bf[co][:], x2sb[:])
            sqt = sq_pool.tile([P, TL], bf16, tag="sqt")
            nc.vector.tensor_mul(sqt[:], x2sb[:], x2sb[:])
            for ch in range(2):
                sl = slice(ch * 512, (ch + 1) * 512)
                nc.tensor.matmul(sx_ch[ch][:], ones_bf[:], x2bf[co][:, sl],
                                 start=(co == 0), stop=(co == NCT - 1))
                nc.tensor.matmul(sq_ch[ch][:], ones_bf[:], sqt[:, sl],
                                 start=(co == 0), stop=(co == NCT - 1))

        mu2 = rows2_pool.tile([1, TL], f32, tag="row")
        ex22 = rows2_pool.tile([1, TL], f32, tag="row")
        for ch in range(2):
            sl = slice(ch * 512, (ch + 1) * 512)
            nc.scalar.mul(mu2[:, sl], sx_ch[ch][:], 1.0 / C)
            nc.scalar.mul(ex22[:, sl], sq_ch[ch][:], 1.0 / C)
        var2 = rows2_pool.tile([1, TL], f32, tag="row")
        nc.vector.tensor_mul(var2[:], mu2[:], mu2[:])
        nc.vector.scalar_tensor_tensor(var2[:], ex22[:], EPS,
                                       var2[:], OP.add, OP.subtract)
        std2 = rows2_pool.tile([1, TL], f32, tag="row")
        nc.scalar.activation(std2[:], var2[:], AF.Sqrt)
        rstd2 = rows2_pool.tile([1, TL], f32, tag="row")
        nc.vector.reciprocal(rstd2[:], std2[:])
        nm2 = rows2_pool.tile([1, TL], f32, tag="row")
        nc.scalar.mul(nm2[:], mu2[:], -1.0)

        xrow2_r = rowr_pool.tile([2, TL], bf16, tag="xrow2b")
        nc.gpsimd.dma_start(xrow2_r[0:1, :], nm2[:])
        nc.gpsimd.dma_start(xrow2_r[1:2, :], std2[:])
        for ch in range(2):
            sl = slice(ch * 512, (ch + 1) * 512)
            bc_ps = bcp_ps2.tile([P, 512], f32, tag="bc")
            nc.tensor.matmul(bc_ps[:], onescol[:], rstd2[:, sl],
                             start=True, stop=True)
            nc.scalar.copy(a2bc[ch][:], bc_ps[:])
        p4.close()

        # =========== Phase 6: FFN (LN folded, rstd2 deferred to ff2 evict) ===========
        p6 = ExitStack()
        w1_pool = p6.enter_context(tc.tile_pool(name="w1", bufs=16))
        ccf_pool = p6.enter_context(tc.tile_pool(name="ccf", bufs=4))
        w2_pool = p6.enter_context(tc.tile_pool(name="w2", bufs=16))
        ffn_ps = p6.enter_context(tc.tile_pool(name="ffnps", bufs=3, space="PSUM"))
        relu_pool = p6.enter_context(tc.tile_pool(name="relu", bufs=1))
        out_pool = p6.enter_context(tc.tile_pool(name="outsb", bufs=4))
        b2_pool = p6.enter_context(tc.tile_pool(name="b2p", bufs=1))

        b2col = [b2_pool.tile([P, 1], f32, name=f"b2c{ci}", tag=f"b2c{ci}")
                 for ci in range(NCT)]
        for ci in range(NCT):
            nc.sync.dma_start(b2col[ci][:], d_b2[ci].unsqueeze(1))

        relu1 = []
        for fog in range(NFT // 4):
            w1t = []
            for ci in range(NCT):
                w = w1_pool.tile([P, 512], bf16, tag="w1t")
                nc.sync.dma_start(w[:], d_w1[ci][:, fog * 512:(fog + 1) * 512])
                w1t.append(w)
            ccf = ccf_pool.tile([2, 512], bf16, tag="ccf")
            nc.sync.dma_start(ccf[:], d_ccf[fog])
            for fol in range(4):
                fo = fog * 4 + fol
                fsl = slice(fol * P, (fol + 1) * P)
                rt = relu_pool.tile([P, TL], bf16, name=f"rl{fo}", tag=f"rl{fo}")
                relu1.append(rt)
                for tc2 in range(2):
                    sl = slice(tc2 * 512, (tc2 + 1) * 512)
                    fp = ffn_ps.tile([P, 512], f32, tag="fp")
                    for ci in range(NCT):
                        nc.tensor.matmul(fp[:], w1t[ci][:, fsl], x2bf[ci][:, sl],
                                         start=(ci == 0), stop=False)
                    nc.tensor.matmul(fp[:], ccf[:, fsl], xrow2_r[:, sl],
                                     start=False, stop=True)
                    nc.scalar.activation(rt[:, sl], fp[:], AF.Relu)

        for co in range(NCT):
            w2g = []
            for gq in range(NFT // 4):
                w = w2_pool.tile([P, 512], bf16, tag="w2t")
                nc.sync.dma_start(w[:], d_w2[co, gq])
                w2g.append(w)
            for tc2 in range(2):
                sl = slice(tc2 * 512, (tc2 + 1) * 512)
                fp = ffn_ps.tile([P, 512], f32, tag="fp")
                for fi in range(NFT):
                    nc.tensor.matmul(fp[:], w2g[fi // 4][:, (fi % 4) * P:(fi % 4 + 1) * P],
                                     relu1[fi][:, sl],
                                     start=(fi == 0), stop=(fi == NFT - 1))
                x2c = out_pool.tile([P, 512], f32, tag="x2c")
                nc.sync.dma_start(x2c[:], x2_dram[co][:, sl])
                tmp = out_pool.tile([P, 512], f32, tag="tmp")
                nc.vector.tensor_mul(tmp[:], fp[:], a2bc[tc2][:])
                osb = out_pool.tile([P, 512], f32, tag="osb")
                nc.vector.scalar_tensor_tensor(osb[:], tmp[:], b2col[co][:],
                                               x2c[:], OP.add, OP.add)
                nc.sync.dma_start(d_out[co][:, sl], osb[:])
        p6.close()
        px2bf.close()
        pattr.close()

    nc.compile()
    return nc


def _prep_inputs(x, Wq, Wk, Wv, Wproj, bproj, W1, b1, W2, b2, g1, beta1, g2, beta2):
    """Build the 8 per-core input maps (host-side sharding + layout prep)."""
    f32 = np.float32
    scale = HS ** -0.5
    x = np.asarray(x, f32)
    Wq = np.asarray(Wq, f32)
    Wk = np.asarray(Wk, f32)
    Wv = np.asarray(Wv, f32)
    Wproj = np.asarray(Wproj, f32)
    W1 = np.asarray(W1, f32)
    b1 = np.asarray(b1, f32)
    W2 = np.asarray(W2, f32)
    g1 = np.asarray(g1, f32)
    beta1 = np.asarray(beta1, f32)
    g2 = np.asarray(g2, f32)
    beta2 = np.asarray(beta2, f32)

    w1g = g2[:, None] * W1
    b1p = b1 + beta2 @ W1
    w1bf = w1g.reshape(NCT, P, F).astype(ml_dtypes.bfloat16)
    ccfbf = np.stack([w1g.sum(0).reshape(NFT // 4, 512),
                      b1p.reshape(NFT // 4, 512)], axis=1).astype(ml_dtypes.bfloat16)
    # [NCT, NFT//4, P, 512]: w2h[co, gq, p, fi*128+cc] = W2[(gq*4+fi)*128+p, co*128+cc]
    w2bf = np.ascontiguousarray(
        W2.reshape(NFT // 4, 4, P, NCT, P).transpose(3, 0, 2, 1, 4).reshape(
            NCT, NFT // 4, P, 512)).astype(ml_dtypes.bfloat16)
    b2r = np.asarray(b2, f32).reshape(NCT, P)
    bprojr = np.asarray(bproj, f32).reshape(NCT, P)
    sp = np.arange(P)[:, None]
    qf = np.arange(512)[None, :]
    m01 = np.stack([(sp + 128 * off <= qf) for off in range(4)]).astype(
        ml_dtypes.bfloat16)
    ident = np.eye(P, dtype=f32)

    xT = [np.ascontiguousarray(x[b].T).reshape(NCT, P, T) for b in range(B)]

    def pair_weights(Wfull, g, scl):
        # raw pair weights [NPAIR, C, P]: cols 0:64 head g*8+2p, 64:128 head +1
        out = np.empty((NPAIR, C, P), f32)
        for p in range(NPAIR):
            hA, hB = g * 8 + 2 * p, g * 8 + 2 * p + 1
            out[p, :, 0:64] = Wfull[hA] * scl
            out[p, :, 64:128] = Wfull[hB] * scl
        return out

    per_g = {}
    for g in range(2):
        d = {}
        wqkv = np.empty((NPAIR, NCT, P, 3 * P), f32)
        ccqkv = np.empty((NPAIR, 2, 3 * P), f32)
        for j, (Wfull, scl) in enumerate(((Wq, scale), (Wk, 1.0), (Wv, 1.0))):
            raw = pair_weights(Wfull, g, scl)      # [NPAIR, C, P]
            wt = g1[None, :, None] * raw           # g1-folded
            wqkv[:, :, :, j * P:(j + 1) * P] = wt.reshape(NPAIR, NCT, P, P)
            ccqkv[:, 0, j * P:(j + 1) * P] = wt.sum(1)
            ccqkv[:, 1, j * P:(j + 1) * P] = np.einsum("c,pcd->pd", beta1, raw)
        d["wqkv"] = wqkv
        d["ccqkv"] = ccqkv
        d["wproj"] = np.ascontiguousarray(
            Wproj[g * 512:(g + 1) * 512]).reshape(NPAIR, P, C).astype(
                ml_dtypes.bfloat16)
        per_g[g] = d

    in_maps = []
    for c in range(8):
        b, g = c // 2, c % 2
        m = {
            "xT": xT[b],
            "xresT": np.ascontiguousarray(xT[b][:, :, g * TL:(g + 1) * TL]),
            "bproj": bprojr,
            "w1": w1bf,
            "ccf": ccfbf,
            "w2": w2bf,
            "b2": b2r,
            "m01": m01,
            "ident": ident,
        }
        m.update(per_g[g])
        in_maps.append(m)
    return in_maps


def kernel(**inputs):
    from concourse.bass_utils import run_bass_kernel_spmd

    if "nc" not in _CACHE:
        _CACHE["nc"] = _build(with_collective=True)
    nc = _CACHE["nc"]
    in_maps = _prep_inputs(**inputs)
    res = run_bass_kernel_spmd(nc, in_maps, list(range(8)))
    out = np.empty((B, T, C), np.float32)
    for c in range(8):
        b, g = c // 2, c % 2
        outT = res.results[c]["outT"].reshape(C, TL)
        out[b, g * TL:(g + 1) * TL, :] = outT.T
    return out
